# revision 1
# baseline (speedup 1.0000x reference)
"""Trainium2 Bass kernel for nn_EDTransformer (encoder-decoder transformer).

Sharding: 8 cores = 4 batch items x 2 sequence halves.
 - Each core owns (item b, half h): computes Q/scores/AV/Wo/MLP/LN for its
   256 local positions, K/V redundantly for the full 512 positions.
 - One 2-core AllGather of fp16 activations per layer (pairs share an item).
 - Unembedding sharded over vocab (4000 rows/core, 32 M-tiles of 125),
   softmax denominator via one 8-core AllReduce of (4,512) partial sums.
Dtypes: fp16 matmul operands (weights pre-transposed+cast on host),
 fp32 PSUM accumulation, fp32 residual stream + LN stats, fp32 output.
"""
import os
import sys

sys.path.insert(0, '/opt/trn_rl_repo')
import numpy as np

import concourse.bacc as bacc
import concourse.tile as tile
import concourse.mybir as mybir
from concourse.bass_utils import run_bass_kernel_spmd

DT = mybir.dt
F16 = DT.float16
F32 = DT.float32

N_CORES = 8
P = 128
DE = 1024          # model dim        (8 ptiles)
KO = DE // P       # 8
DMLP = 4096        # mlp dim          (32 ptiles)
MO = DMLP // P     # 32
H = 16             # heads
DA = 64            # attn dim per head
L = 512            # sequence length
LL = 256           # local positions per core
NV = 32000
NVC = NV // N_CORES  # 4000 vocab rows per core
UM = 125           # vocab M-tile
UMT = NVC // UM    # 32 M-tiles
LENC = 2
LDEC = 2
EPS = 1e-5

PAIR_GROUPS = [[0, 1], [2, 3], [4, 5], [6, 7]]
ALL_GROUP = [list(range(N_CORES))]

_CACHE = {}


# ----------------------------------------------------------------------------
# device program
# ----------------------------------------------------------------------------

def _attn(nc, tc, pools, Eres32, qin16, kvin16, wq_d, wk_d, wv_d, wo_d,
          mask, name):
    """One multi-head attention block; accumulates Wo output into Eres32.

    qin16  : [128, KO, LL] fp16  local stream (query input)
    kvin16 : [128, KO, L]  fp16  full-sequence stream (key/value input)
    wq_d/wk_d/wv_d/wo_d: dram APs [128, KO, 1024] (pre-transposed; wq scaled)
    mask   : [128, 4, LL] fp16 sbuf tile or None
    """
    sb = pools['att']
    p256 = pools['p256']
    p512 = pools['p512']
    ones = pools['ones']
    KT = L // P  # 4 kz tiles

    # q: [128(2h x 64a), pr, LL]
    q16 = sb.tile([P, KO, LL], F16, tag='q16')
    for pr in range(KO):
        wt = pools['wqp'].tile([P, KO, P], F16, tag='wqt')
        nc.sync.dma_start(wt[:], wq_d[:, :, pr * P:(pr + 1) * P])
        ps = p256.tile([P, LL], F32, tag='p256')
        for k in range(KO):
            nc.tensor.matmul(ps[:], wt[:, k, :],
                             qin16[:, k, :], start=(k == 0), stop=(k == KO - 1))
        nc.vector.tensor_copy(q16[:, pr, :], ps[:])
    # k: [128(2h x 64a), pr, L]
    k16 = sb.tile([P, KO, L], F16, tag='k16')
    for pr in range(KO):
        wt = pools['wkp'].tile([P, KO, P], F16, tag='wkt')
        nc.sync.dma_start(wt[:], wk_d[:, :, pr * P:(pr + 1) * P])
        ps = p512.tile([P, L], F32, tag='p512')
        for k in range(KO):
            nc.tensor.matmul(ps[:], wt[:, k, :],
                             kvin16[:, k, :], start=(k == 0), stop=(k == KO - 1))
        nc.vector.tensor_copy(k16[:, pr, :], ps[:])
    # vT: [128(kz), kt, 1024(h*64+o)]
    vt16 = sb.tile([P, KT, H * DA], F16, tag='vt16')
    for nch in range(2):
        wt = pools['wvp'].tile([P, KO, 512], F16, tag='wvt')
        nc.sync.dma_start(wt[:], wv_d[:, :, nch * 512:(nch + 1) * 512])
        for kt in range(KT):
            ps = p512.tile([P, 512], F32, tag='p512')
            for k in range(KO):
                nc.tensor.matmul(ps[:], kvin16[:, k, kt * P:(kt + 1) * P],
                                 wt[:, k, :],
                                 start=(k == 0), stop=(k == KO - 1))
            nc.vector.tensor_copy(vt16[:, kt, nch * 512:(nch + 1) * 512], ps[:])

    # scores -> exp -> (mask) ; exp16: [128(kz), h, kt, LL]
    exp16 = sb.tile([P, H, KT, LL], F16, tag='exp16')
    for h in range(H):
        pr, hp = h // 2, (h % 2) * DA
        for kt in range(KT):
            ps = p256.tile([P, LL], F32, tag='p256')
            nc.tensor.matmul(ps[:], k16[hp:hp + DA, pr, kt * P:(kt + 1) * P],
                             q16[hp:hp + DA, pr, :], start=True, stop=True)
            nc.scalar.activation(exp16[:, h, kt, :], ps[:],
                                 mybir.ActivationFunctionType.Exp)
    if mask is not None:
        for kt in range(KT):
            nc.vector.tensor_tensor(
                exp16[:, :, kt, :], exp16[:, :, kt, :],
                mask[:, kt, None, :].to_broadcast((P, H, LL)),
                mybir.AluOpType.mult)

    # AV with 2-head column packing + per-head normalize.
    # deno: ones-matmul with M=128 -> every partition row holds the colsums.
    y16 = sb.tile([P, KO, LL], F16, tag='y16')
    for pr in range(KO):
        hA, hB = 2 * pr, 2 * pr + 1
        pd = p512.tile([P, 2 * LL], F32, tag='p512')
        for kt in range(KT):
            nc.tensor.matmul(pd[:], ones[:, :],
                             exp16[:, hA:hB + 1, kt, :],
                             start=(kt == 0), stop=(kt == KT - 1))
        ysc = sb.tile([P, 2, LL], F32, tag='ysc')
        nc.vector.reciprocal(ysc[:], pd[:])
        ps = p256.tile([P, LL], F32, tag='p256')
        for kt in range(KT):
            nc.tensor.matmul(ps[:DA, :], vt16[:, kt, hA * DA:(hA + 1) * DA],
                             exp16[:, hA, kt, :], start=(kt == 0),
                             stop=(kt == KT - 1), tile_position=(0, 0))
            nc.tensor.matmul(ps[DA:, :], vt16[:, kt, hB * DA:(hB + 1) * DA],
                             exp16[:, hB, kt, :], start=(kt == 0),
                             stop=(kt == KT - 1), tile_position=(0, DA))
        nc.vector.tensor_tensor(y16[:DA, pr, :], ps[:DA, :], ysc[:DA, 0, :],
                                mybir.AluOpType.mult)
        nc.vector.tensor_tensor(y16[DA:, pr, :], ps[DA:, :], ysc[DA:, 1, :],
                                mybir.AluOpType.mult)

    # Wo -> accumulate into residual
    for dt in range(KO):
        wt = pools['wop'].tile([P, KO, P], F16, tag='wot')
        nc.sync.dma_start(wt[:], wo_d[:, :, dt * P:(dt + 1) * P])
        ps = p256.tile([P, LL], F32, tag='p256')
        for k in range(KO):
            nc.tensor.matmul(ps[:], wt[:, k, :],
                             y16[:, k, :], start=(k == 0), stop=(k == KO - 1))
        nc.vector.tensor_tensor(Eres32[:, dt, :], Eres32[:, dt, :], ps[:],
                                mybir.AluOpType.add)
    tp = pools.get('tapfn')
    if tp:
        tp(f'{name}_q', q16); tp(f'{name}_k', k16); tp(f'{name}_vt', vt16)
        tp(f'{name}_exp', exp16); tp(f'{name}_y', y16)


def _mlp(nc, tc, pools, Eres32, ein16, w1_dram, w2_dram, name):
    p256 = pools['p256']
    h16 = pools['mlp'].tile([P, MO, LL], F16, tag='h16')
    for mt in range(MO):
        w1t = pools['w1p'].tile([P, KO, P], F16, tag='w1t')
        nc.sync.dma_start(w1t[:], w1_dram[:, :, mt * P:(mt + 1) * P])
        ps = p256.tile([P, LL], F32, tag='p256')
        for k in range(KO):
            nc.tensor.matmul(ps[:], w1t[:, k, :], ein16[:, k, :],
                             start=(k == 0), stop=(k == KO - 1))
        nc.scalar.activation(h16[:, mt, :], ps[:],
                             mybir.ActivationFunctionType.Relu)
    for dt in range(KO):
        w2t = pools['w2p'].tile([P, MO, P], F16, tag='w2t')
        nc.sync.dma_start(w2t[:], w2_dram[:, :, dt * P:(dt + 1) * P])
        ps = p256.tile([P, LL], F32, tag='p256')
        for k in range(MO):
            nc.tensor.matmul(ps[:], w2t[:, k, :], h16[:, k, :],
                             start=(k == 0), stop=(k == MO - 1))
        nc.vector.tensor_tensor(Eres32[:, dt, :], Eres32[:, dt, :], ps[:],
                                mybir.AluOpType.add)


def _ln(nc, tc, pools, Eres32, e16out, name):
    """In-place layernorm over features; writes fp16 copy to e16out."""
    p256 = pools['p256']
    ones = pools['ones']
    stat = pools['stat']

    e16pre = pools['lnp'].tile([P, KO, LL], F16, tag='e16pre')
    nc.vector.tensor_copy(e16pre[:], Eres32[:])
    sq16 = pools['lnp'].tile([P, KO, LL], F16, tag='sq16')
    nc.vector.tensor_tensor(sq16[:], e16pre[:], e16pre[:],
                            mybir.AluOpType.mult)
    # sums with M=128 ones -> replicated rows; stats stay [128, LL]
    pss = p256.tile([P, LL], F32, tag='p256')
    psq = p256.tile([P, LL], F32, tag='p256')
    for k in range(KO):
        nc.tensor.matmul(pss[:], ones[:, :], e16pre[:, k, :],
                         start=(k == 0), stop=(k == KO - 1))
    for k in range(KO):
        nc.tensor.matmul(psq[:], ones[:, :], sq16[:, k, :],
                         start=(k == 0), stop=(k == KO - 1))
    mean = stat.tile([P, LL], F32, tag='mean')
    nc.vector.tensor_scalar_mul(mean[:], pss[:], 1.0 / DE)
    var = stat.tile([P, LL], F32, tag='var')
    nc.vector.tensor_scalar_mul(var[:], psq[:], 1.0 / DE)
    msq = stat.tile([P, LL], F32, tag='msq')
    nc.vector.tensor_tensor(msq[:], mean[:], mean[:], mybir.AluOpType.mult)
    nc.vector.tensor_tensor(var[:], var[:], msq[:], mybir.AluOpType.subtract)
    nc.vector.tensor_scalar_mul(var[:], var[:], float(DE) / (DE - 1))
    std = stat.tile([P, LL], F32, tag='std')
    nc.scalar.activation(std[:], var[:], mybir.ActivationFunctionType.Sqrt,
                         bias=pools['eps128'])
    inv = stat.tile([P, LL], F32, tag='inv')
    nc.vector.reciprocal(inv[:], std[:])
    negms = stat.tile([P, LL], F32, tag='negms')
    nc.vector.tensor_tensor(negms[:], mean[:], inv[:], mybir.AluOpType.mult)
    nc.vector.tensor_scalar_mul(negms[:], negms[:], -1.0)
    nc.vector.tensor_tensor(
        Eres32[:], Eres32[:],
        inv[:, None, :].to_broadcast((P, KO, LL)), mybir.AluOpType.mult)
    nc.vector.tensor_tensor(
        Eres32[:], Eres32[:],
        negms[:, None, :].to_broadcast((P, KO, LL)), mybir.AluOpType.add)
    nc.vector.tensor_copy(e16out[:], Eres32[:])
    tp = pools.get('tapfn')
    if tp:
        tp(f'{name}_out', Eres32)


def _allgather_pair(nc, tc, pools, e16loc, full16, agin, agout, tag):
    """e16loc [128, KO, LL] -> pair AllGather -> full16 [128, KO, L]."""
    nc.gpsimd.dma_start(agin[:], e16loc[:])
    nc.gpsimd.collective_compute(
        "AllGather", mybir.AluOpType.bypass,
        ins=[agin[:]], outs=[agout[:]],
        replica_groups=PAIR_GROUPS)
    nc.gpsimd.dma_start(
        full16[:].rearrange('ki ko (r p) -> ki ko r p', r=2),
        agout[:].rearrange('r ki ko p -> ki ko r p'))


def build_program(taps=()):
    taps = set(taps)
    nc = bacc.Bacc("TRN2", target_bir_lowering=False, debug=False,
                   num_devices=N_CORES)

    # ---- dram inputs ----
    din = {}
    def dram_in(nm, shape, dt=F16):
        din[nm] = nc.dram_tensor(nm, list(shape), dt, kind="ExternalInput")
        return din[nm]

    z0f = dram_in('z0_full16', [P, KO, L])
    x0f = dram_in('x0_full16', [P, KO, L])
    z0l32 = dram_in('z0_loc32', [P, KO, LL], F32)
    x0l32 = dram_in('x0_loc32', [P, KO, LL], F32)
    z0l16 = dram_in('z0_loc16', [P, KO, LL])
    x0l16 = dram_in('x0_loc16', [P, KO, LL])
    mask_self = dram_in('mask_self', [P, 4, LL])
    for pfx, nl in (('enc', LENC), ('dec', LDEC)):
        for w in ('wqT', 'wkT', 'wvT', 'woT'):
            dram_in(f'{pfx}_{w}', [nl, P, KO, DE])
        dram_in(f'{pfx}_w1T', [nl, P, KO, DMLP])
        dram_in(f'{pfx}_w2T', [nl, P, MO, DE])
    wuT = dram_in('wuT', [P, KO, NVC])

    outp = nc.dram_tensor('outp', [UMT, UM, 4, L], F32, kind="ExternalOutput")

    # internal dram for collectives
    agin = nc.dram_tensor('agin', [P, KO, LL], F16)
    agout = nc.dram_tensor('agout', [2, P, KO, LL], F16)
    ag8in = nc.dram_tensor('ag8in', [P, KO, LL], F16)
    ag8out = nc.dram_tensor('ag8out', [N_CORES, P, KO, LL], F16,
                            addr_space='Shared')
    arin = nc.dram_tensor('arin', [1, 4, L], F32)
    arout = nc.dram_tensor('arout', [1, 4, L], F32, addr_space="Shared")
    dspd = nc.dram_tensor('dspd', [1, 4, L], F32)


    import contextlib
    with tile.TileContext(nc) as tc, contextlib.ExitStack() as octx:
        const = octx.enter_context(tc.tile_pool(name='const', bufs=1))
        ones = const.tile([P, P], F16)
        nc.vector.memset(ones[:], 1.0)
        eps1 = const.tile([1, 1], F32)
        nc.vector.memset(eps1[:], EPS)
        eps128 = const.tile([P, 1], F32)
        nc.vector.memset(eps128[:], EPS)
        ones32 = const.tile([1, P], F32)
        nc.vector.memset(ones32[:], 1.0)
        msk = const.tile([P, 4, LL], F16)
        nc.sync.dma_start(msk[:], mask_self[:])

        # ================= layer phase =================
        with contextlib.ExitStack() as ctx:
            stream = ctx.enter_context(tc.tile_pool(name='stream', bufs=1))
            att = ctx.enter_context(tc.tile_pool(name='att', bufs=1))
            mlpp = ctx.enter_context(tc.tile_pool(name='mlpp', bufs=1))
            lnp = ctx.enter_context(tc.tile_pool(name='lnp', bufs=1))
            stat = ctx.enter_context(tc.tile_pool(name='stat', bufs=1))
            wqp = ctx.enter_context(tc.tile_pool(name='wqp', bufs=3))
            wkp = ctx.enter_context(tc.tile_pool(name='wkp', bufs=3))
            wvp = ctx.enter_context(tc.tile_pool(name='wvp', bufs=2))
            wop = ctx.enter_context(tc.tile_pool(name='wop', bufs=3))
            w1p = ctx.enter_context(tc.tile_pool(name='w1p', bufs=4))
            w2p = ctx.enter_context(tc.tile_pool(name='w2p', bufs=2))
            p256 = ctx.enter_context(tc.tile_pool(name='p256', bufs=5,
                                                  space='PSUM'))
            p512 = ctx.enter_context(tc.tile_pool(name='p512', bufs=3,
                                                  space='PSUM'))

            pools = dict(att=att, mlp=mlpp, lnp=lnp, p256=p256, p512=p512,
                         stat=stat, ones=ones, ones32=ones32,
                         eps1=eps1, eps128=eps128[:], wqp=wqp, wkp=wkp,
                         wvp=wvp, wop=wop, w1p=w1p, w2p=w2p)

            def tapfn(nm, t):
                if nm not in taps:
                    return
                d = nc.dram_tensor('tap_' + nm, list(t.shape),
                                   t.dtype, kind="ExternalOutput")
                nc.sync.dma_start(d[:], t[:])
            pools['tapfn'] = tapfn

            # ======== encoder ========
            Eres = stream.tile([P, KO, LL], F32, tag='res')
            nc.sync.dma_start(Eres[:], z0l32[:])
            Zfull = stream.tile([P, KO, L], F16, tag='Zfull')
            nc.sync.dma_start(Zfull[:], z0f[:])
            eloc = stream.tile([P, KO, LL], F16, tag='loc_a')
            nc.sync.dma_start(eloc[:], z0l16[:])

            for l in range(LENC):
                _attn(nc, tc, pools, Eres, eloc, Zfull,
                      din['enc_wqT'][l], din['enc_wkT'][l],
                      din['enc_wvT'][l], din['enc_woT'][l], None, f'e{l}a')
                eloc = stream.tile([P, KO, LL], F16, tag='loc_b')
                _ln(nc, tc, pools, Eres, eloc, f'e{l}ln1')
                _mlp(nc, tc, pools, Eres, eloc, din['enc_w1T'][l],
                     din['enc_w2T'][l], f'e{l}m')
                eloc = stream.tile([P, KO, LL], F16, tag='loc_a')
                _ln(nc, tc, pools, Eres, eloc, f'e{l}ln2')
                Zfull = stream.tile([P, KO, L], F16, tag='Zfull')
                _allgather_pair(nc, tc, pools, eloc, Zfull, agin, agout,
                                f'e{l}')


            # ======== decoder ========
            Eres = stream.tile([P, KO, LL], F32, tag='res')
            nc.sync.dma_start(Eres[:], x0l32[:])
            Xfull = stream.tile([P, KO, L], F16, tag='Xfull')
            nc.sync.dma_start(Xfull[:], x0f[:])
            eloc = stream.tile([P, KO, LL], F16, tag='loc_a')
            nc.sync.dma_start(eloc[:], x0l16[:])

            for l in range(LDEC):
                _attn(nc, tc, pools, Eres, eloc, Xfull,
                      din['dec_wqT'][l], din['dec_wkT'][l],
                      din['dec_wvT'][l], din['dec_woT'][l], msk, f'd{l}s')
                eloc = stream.tile([P, KO, LL], F16, tag='loc_b')
                _ln(nc, tc, pools, Eres, eloc, f'd{l}ln1')
                _attn(nc, tc, pools, Eres, eloc, Zfull,
                      din['dec_wqT'][l], din['dec_wkT'][l],
                      din['dec_wvT'][l], din['dec_woT'][l], None, f'd{l}c')
                eloc = stream.tile([P, KO, LL], F16, tag='loc_c')
                _ln(nc, tc, pools, Eres, eloc, f'd{l}ln2')
                _mlp(nc, tc, pools, Eres, eloc, din['dec_w1T'][l],
                     din['dec_w2T'][l], f'd{l}m')
                eloc = stream.tile([P, KO, LL], F16, tag='loc_a')
                _ln(nc, tc, pools, Eres, eloc, f'd{l}ln3')
                if l < LDEC - 1:
                    Xfull = stream.tile([P, KO, L], F16, tag='Xfull')
                    _allgather_pair(nc, tc, pools, eloc, Xfull, agin, agout,
                                    f'd{l}')


            # send final local activations for the 8-core gather
            nc.gpsimd.dma_start(ag8in[:], eloc[:])

        # ================= unembed phase =================
        nc.gpsimd.collective_compute(
            "AllGather", mybir.AluOpType.bypass,
            ins=[ag8in[:]], outs=[ag8out[:]], replica_groups=ALL_GROUP)

        with contextlib.ExitStack() as ctx:
            usb = ctx.enter_context(tc.tile_pool(name='usb', bufs=1))
            ures_p = ctx.enter_context(tc.tile_pool(name='ures_p', bufs=2))
            wup = ctx.enter_context(tc.tile_pool(name='wup', bufs=3))
            u512 = ctx.enter_context(tc.tile_pool(name='u512', bufs=4,
                                                  space='PSUM'))
            udeno = ctx.enter_context(tc.tile_pool(name='udeno', bufs=1,
                                                   space='PSUM'))

            XF = usb.tile([P, KO, 4 * L], F16, tag='XF')
            nc.sync.dma_start(
                XF[:].rearrange('ki ko (r p) -> ki ko r p', r=N_CORES),
                ag8out[:].rearrange('r ki ko p -> ki ko r p'))

            if 'xf' in taps:
                d = nc.dram_tensor('tap_xf', [P, KO, 4 * L], F16,
                                   kind="ExternalOutput")
                nc.sync.dma_start(d[:], XF[:])
            expu = usb.tile([P, UMT, 4, L], F16, tag='expu')
            pdeno = [udeno.tile([1, L], F32, tag=f'pdeno{j}',
                                name=f'pdeno{j}') for j in range(4)]
            for mt in range(UMT):
                wut = wup.tile([P, KO, UM], F16, tag='wut')
                nc.sync.dma_start(wut[:], wuT[:, :, mt * UM:(mt + 1) * UM])
                pss = [u512.tile([P, 512], F32, tag='u512', name=f'ups{j}')
                       for j in range(4)]
                # k-outer: each lhsT load serves 4 matmuls
                for k in range(KO):
                    for j in range(4):
                        nc.tensor.matmul(pss[j][:UM, :], wut[:, k, :],
                                         XF[:, k, j * L:(j + 1) * L],
                                         start=(k == 0), stop=(k == KO - 1))
                for j in range(4):
                    nc.scalar.activation(expu[:UM, mt, j, :], pss[j][:UM, :],
                                         mybir.ActivationFunctionType.Exp)
                    nc.tensor.matmul(pdeno[j][:], ones[:UM, :1],
                                     expu[:UM, mt, j, :],
                                     start=(mt == 0), stop=(mt == UMT - 1))
            deno = usb.tile([1, 4, L], F32, tag='deno')
            for j in range(4):
                nc.vector.tensor_copy(deno[:, j, :], pdeno[j][:])
            nc.gpsimd.dma_start(arin[:], deno[:])
            nc.gpsimd.collective_compute(
                "AllReduce", mybir.AluOpType.add,
                ins=[arin[:]], outs=[arout[:]], replica_groups=ALL_GROUP)
            if 'deno' in taps:
                d = nc.dram_tensor('tap_deno', [1, 4, L], F32,
                                   kind="ExternalOutput")
                nc.sync.dma_start(d[:], deno[:])
            if 'expu' in taps:
                d = nc.dram_tensor('tap_expu', [P, 4, 4, L], F16,
                                   kind="ExternalOutput")
                nc.sync.dma_start(d[:], expu[:, 0:4, :, :])
            # spread the 2048 reciprocals across partitions (via dram views)
            dsp = usb.tile([P, 16], F32, tag='dsp')  # 128 part x 16
            nc.sync.dma_start(
                dsp[:], arout[:].rearrange('o a (p n) -> (o a p) n', p=32))
            nc.vector.reciprocal(dsp[:], dsp[:])
            nc.sync.dma_start(
                dspd[:].rearrange('o a (p n) -> (o a p) n', p=32), dsp[:])
            nc.sync.dma_start(deno[:], dspd[:])
            binv = usb.tile([P, 4, L], F32, tag='binv')
            for j in range(4):
                pb = u512.tile([P, L], F32, tag='u512')
                nc.tensor.matmul(pb[:], ones32[:1, :], deno[:, j, :],
                                 start=True, stop=True)
                nc.vector.tensor_copy(binv[:, j, :], pb[:])
            dmae = [nc.sync, nc.gpsimd, nc.scalar, nc.gpsimd]
            for mt in range(UMT):
                res = ures_p.tile([P, 4, L], F32, tag='ures')
                nc.vector.tensor_tensor(res[:UM], expu[:UM, mt], binv[:UM],
                                        mybir.AluOpType.mult)
                for j in range(4):
                    dmae[j].dma_start(outp[mt, :, j], res[:UM, j])

    nc.compile()
    return nc


# ----------------------------------------------------------------------------
# host-side prep
# ----------------------------------------------------------------------------

def _to_kimaj(a):
    """[K, M] -> [128, K//128, M] with K = ko*128 + ki."""
    K, M = a.shape
    return np.ascontiguousarray(
        a.reshape(K // P, P, M).transpose(1, 0, 2))


def prep_inputs(inputs):
    f = lambda k: np.asarray(inputs[k], dtype=np.float32)
    We, Wp, Wu = f('We'), f('Wp'), f('Wu')
    x = np.asarray(inputs['x']).astype(np.int64)
    z = np.asarray(inputs['z']).astype(np.int64)

    shared = {}
    for pfx, nl in (('enc', LENC), ('dec', LDEC)):
        Wq, Wk, Wv = f(pfx + '_Wq'), f(pfx + '_Wk'), f(pfx + '_Wv')
        Wo, W1, W2 = f(pfx + '_Wo'), f(pfx + '_W1'), f(pfx + '_W2')
        wq, wk, wv, wo, w1, w2 = [], [], [], [], [], []
        for l in range(nl):
            qa = Wq[l].transpose(2, 0, 1).reshape(DE, H * DA) * (DA ** -0.5)
            ka = Wk[l].transpose(2, 0, 1).reshape(DE, H * DA)
            va = Wv[l].transpose(2, 0, 1).reshape(DE, H * DA)
            wq.append(_to_kimaj(qa)); wk.append(_to_kimaj(ka))
            wv.append(_to_kimaj(va))
            wo.append(_to_kimaj(Wo[l].T))
            w1.append(_to_kimaj(W1[l].T))
            w2.append(_to_kimaj(W2[l].T))
        shared[f'{pfx}_wqT'] = np.stack(wq).astype(np.float16)
        shared[f'{pfx}_wkT'] = np.stack(wk).astype(np.float16)
        shared[f'{pfx}_wvT'] = np.stack(wv).astype(np.float16)
        shared[f'{pfx}_woT'] = np.stack(wo).astype(np.float16)
        shared[f'{pfx}_w1T'] = np.stack(w1).astype(np.float16)
        shared[f'{pfx}_w2T'] = np.stack(w2).astype(np.float16)

    pos = Wp[:L]  # [512, 1024]
    in_maps = []
    for c in range(N_CORES):
        b, h = c // 2, c % 2
        m = dict(shared)
        for nm, tok in (('z0', z[b]), ('x0', x[b])):
            E0 = (We[tok] + pos).T.astype(np.float32)      # [1024, 512]
            E0k = E0.reshape(KO, P, L)                     # [ko, ki, p]
            m[nm + '_full16'] = np.ascontiguousarray(
                E0k.transpose(1, 0, 2)).astype(np.float16)
            loc = E0k[:, :, h * LL:(h + 1) * LL].transpose(1, 0, 2)
            m[nm + '_loc32'] = np.ascontiguousarray(loc)
            m[nm + '_loc16'] = np.ascontiguousarray(loc).astype(np.float16)
        kglob = np.arange(L)[:, None]
        qglob = (h * LL + np.arange(LL))[None, :]
        msk = (kglob <= qglob).astype(np.float16)          # [512, 256]
        m['mask_self'] = np.ascontiguousarray(
            msk.reshape(4, P, LL).transpose(1, 0, 2))
        wus = Wu[c * NVC:(c + 1) * NVC].T                  # [1024, 4000]
        m['wuT'] = _to_kimaj(wus).astype(np.float16)
        in_maps.append(m)
    return in_maps


def assemble(results):
    """results: list of per-core dicts with 'outp' [UMT, UM, 4, L] fp32."""
    out = np.empty((4, NV, L), dtype=np.float32)
    for c, r in enumerate(results):
        o = r['outp']                                     # [32, 125, 4, 512]
        o = o.reshape(NVC, 4, L).transpose(1, 0, 2)       # [4, 4000, 512]
        out[:, c * NVC:(c + 1) * NVC, :] = o
    return out


def run(inputs, trace=False, taps=(), trace_kwargs=None):
    key = ('prog', tuple(sorted(taps)))
    if key not in _CACHE:
        _CACHE[key] = build_program(taps=taps)
    nc = _CACHE[key]
    in_maps = prep_inputs(inputs)
    res = run_bass_kernel_spmd(nc, in_maps, list(range(N_CORES)),
                               trace=trace, **(trace_kwargs or {}))
    return res


def kernel(**inputs):
    res = run(inputs, trace=False)
    return assemble(res.results)



# revision 13
# speedup vs baseline: 1.1563x; 1.1563x over previous
"""Trainium2 Bass kernel for nn_EDTransformer (encoder-decoder transformer).

Sharding: 8 cores = 4 batch items x 2 sequence halves.
 - Each core owns (item b, half h): computes Q/scores/AV/Wo/MLP/LN for its
   256 local positions; K/V computed for the LOCAL half only and completed
   via a 2-core AllGather of K/V per attention block.
 - Decoder self+cross attention share one weight load per layer.
 - Unembedding sharded over vocab (4000 rows/core, 8 chunks of 500),
   computed TRANSPOSED (positions on partitions) so the softmax denominator
   comes from the Act engine accumulator and the normalize is a
   per-partition scale; denominator summed via one 8-core AllReduce.
 - Weights pre-tiled host-side so every DMA reads contiguous >=2KB runs
   per partition; weight loads spread across sync/scalar queues.
Dtypes: fp16 matmul operands, fp32 PSUM, fp32 residual + LN stats,
 fp16 output (cast to fp32 on host).
"""
import os
import sys

sys.path.insert(0, '/opt/trn_rl_repo')
import numpy as np

import concourse.bacc as bacc
import concourse.tile as tile
import concourse.mybir as mybir
from concourse.bass_utils import run_bass_kernel_spmd

DT = mybir.dt
F16 = DT.float16
F32 = DT.float32
AF = mybir.ActivationFunctionType

N_CORES = 8
P = 128
DE = 1024           # model dim (8 ptiles)
KO = DE // P        # 8
DMLP = 4096         # mlp dim
MO = DMLP // P      # 32
H = 16              # heads
DA = 64             # attn dim per head
L = 512             # sequence length
LL = 256            # local positions per core
KT = L // P         # 4 key tiles
NV = 32000
NVC = NV // N_CORES  # 4000 vocab rows per core
VC = 500            # vocab chunk (8 chunks of 500)
NVCH = NVC // VC    # 8
LENC = 2
LDEC = 2
EPS = 1e-5

PAIR_GROUPS = [[0, 1], [2, 3], [4, 5], [6, 7]]
ALL_GROUP = [list(range(N_CORES))]

_CACHE = {}


# ----------------------------------------------------------------------------
# device program
# ----------------------------------------------------------------------------

def _kv_proj_ag(nc, pools, W, kvin16, agin, agout, kfull, vt16):
    """Project K/V from local stream and pair-AllGather to full length.

    kvin16: [128, KO, LL] local stream.
    kfull : [128, KO, L]  (partitions = 2h x 64a rows)
    vt16  : [128, KT, H*DA] (partitions = key positions)
    """
    p256 = pools['p256']
    p512 = pools['p512']
    kloc = pools['att'].tile([P, KO, LL], F16, tag='kloc')
    for pr in range(KO):
        ps = p256.tile([P, LL], F32, tag='p256')
        for k in range(KO):
            nc.tensor.matmul(ps[:], W['wk'][:, k, pr * P:(pr + 1) * P],
                             kvin16[:, k, :], start=(k == 0), stop=(k == KO - 1))
        nc.vector.tensor_copy(kloc[:, pr, :], ps[:])
    vloc = pools['att'].tile([P, 2, H * DA], F16, tag='vloc')
    for lc in range(2):
        for nch in range(2):
            ps = p512.tile([P, 512], F32, tag='p512', bufs=2)
            for k in range(KO):
                nc.tensor.matmul(ps[:], kvin16[:, k, lc * P:(lc + 1) * P],
                                 W['wv'][:, k, nch * 512:(nch + 1) * 512],
                                 start=(k == 0), stop=(k == KO - 1))
            nc.vector.tensor_copy(vloc[:, lc, nch * 512:(nch + 1) * 512], ps[:])
    # stage k (2048) + v (2048) into one dram buffer, AllGather over the pair
    nc.gpsimd.dma_start(
        agin[:, 0:2048].rearrange('p (a b) -> p a b', a=KO), kloc[:])
    nc.gpsimd.dma_start(
        agin[:, 2048:4096].rearrange('p (a b) -> p a b', a=2), vloc[:])
    nc.gpsimd.collective_compute(
        "AllGather", mybir.AluOpType.bypass,
        ins=[agin[:]], outs=[agout[:]], replica_groups=PAIR_GROUPS)
    for r in range(2):
        nc.gpsimd.dma_start(
            kfull[:, :, r * LL:(r + 1) * LL],
            agout[r, :, 0:2048].rearrange('p (a b) -> p a b', a=KO))
        nc.gpsimd.dma_start(
            vt16[:, r * 2:(r + 1) * 2, :],
            agout[r, :, 2048:4096].rearrange('p (a b) -> p a b', a=2))


def _attn_core(nc, pools, Eres32, qin16, W, kfull, vt16, mask, name):
    """Q projection, scores/softmax/AV per head-pair, Wo accumulate."""
    p256 = pools['p256']
    p512 = pools['p512']
    ones = pools['ones']
    sb = pools['att']

    q16 = sb.tile([P, KO, LL], F16, tag='q16')
    for pr in range(KO):
        ps = p256.tile([P, LL], F32, tag='p256')
        for k in range(KO):
            nc.tensor.matmul(ps[:], W['wq'][:, k, pr * P:(pr + 1) * P],
                             qin16[:, k, :], start=(k == 0), stop=(k == KO - 1))
        nc.vector.tensor_copy(q16[:, pr, :], ps[:])

    y16 = sb.tile([P, KO, LL], F16, tag='y16')
    for pr in range(KO):
        hA, hB = 2 * pr, 2 * pr + 1
        # scores -> exp, 2 heads x 4 kt; psum pairs give [128, 512] exps
        exp16 = sb.tile([P, 2, KT, LL], F16, tag='exp16', bufs=2)
        for hh in range(2):
            h = hA + hh
            hp = (h % 2) * DA
            for kp in range(2):
                ps = p512.tile([P, 2, LL], F32, tag='psc', bufs=2)
                for ki in range(2):
                    kt = 2 * kp + ki
                    nc.tensor.matmul(
                        ps[:, ki, :],
                        kfull[hp:hp + DA, pr, kt * P:(kt + 1) * P],
                        q16[hp:hp + DA, pr, :], start=True, stop=True)
                nc.scalar.activation(exp16[:, hh, 2 * kp:2 * kp + 2, :],
                                     ps[:], AF.Exp)
        if mask is not None:
            nc.vector.tensor_tensor(
                exp16[:], exp16[:],
                mask[:, None, :, :].to_broadcast((P, 2, KT, LL)),
                mybir.AluOpType.mult)
        # denominators (replicated over partitions via ones matmul)
        pd = p512.tile([P, 2, LL], F32, tag='pd', bufs=2)
        for kt in range(KT):
            nc.tensor.matmul(pd[:], ones[:, :], exp16[:, :, kt, :],
                             start=(kt == 0), stop=(kt == KT - 1))
        ysc = pools['stat'].tile([P, 2, LL], F32, tag='ysc', bufs=2)
        nc.vector.reciprocal_approx_fast(ysc[:], pd[:])
        # AV (2-head column packing)
        ps = p256.tile([P, LL], F32, tag='p256')
        for kt in range(KT):
            nc.tensor.matmul(ps[:DA, :], vt16[:, kt, hA * DA:(hA + 1) * DA],
                             exp16[:, 0, kt, :], start=(kt == 0),
                             stop=(kt == KT - 1), tile_position=(0, 0))
            nc.tensor.matmul(ps[DA:, :], vt16[:, kt, hB * DA:(hB + 1) * DA],
                             exp16[:, 1, kt, :], start=(kt == 0),
                             stop=(kt == KT - 1), tile_position=(0, DA))
        nc.vector.tensor_tensor(y16[:DA, pr, :], ps[:DA, :], ysc[:DA, 0, :],
                                mybir.AluOpType.mult)
        nc.vector.tensor_tensor(y16[DA:, pr, :], ps[DA:, :], ysc[DA:, 1, :],
                                mybir.AluOpType.mult)

    # Wo -> accumulate into residual
    for dt in range(KO):
        ps = p256.tile([P, LL], F32, tag='p256')
        for k in range(KO):
            nc.tensor.matmul(ps[:], W['wo'][:, k, dt * P:(dt + 1) * P],
                             y16[:, k, :], start=(k == 0), stop=(k == KO - 1))
        nc.vector.tensor_tensor(Eres32[:, dt, :], Eres32[:, dt, :], ps[:],
                                mybir.AluOpType.add)
    tp = pools.get('tapfn')
    if tp:
        tp(f'{name}_q', q16)
        tp(f'{name}_y', y16)


def _load_attn_w(nc, pools, wq_d, wk_d, wv_d, wo_d):
    wq = pools['wqp'].tile([P, KO, DE], F16, tag='wq')
    nc.sync.dma_start(wq[:], wq_d[:])
    wk = pools['wkp'].tile([P, KO, DE], F16, tag='wk')
    nc.scalar.dma_start(wk[:], wk_d[:])
    wv = pools['wvp'].tile([P, KO, DE], F16, tag='wv')
    nc.sync.dma_start(wv[:], wv_d[:])
    wo = pools['wop'].tile([P, KO, DE], F16, tag='wo')
    nc.scalar.dma_start(wo[:], wo_d[:])
    return {'wq': wq, 'wk': wk, 'wv': wv, 'wo': wo}


def _mlp(nc, pools, Eres32, ein16, w1_d, w2_d, name):
    p256 = pools['p256']
    h16 = pools['mlp'].tile([P, MO, LL], F16, tag='h16')
    for c in range(8):
        w1t = pools['w1p'].tile([P, KO, 512], F16, tag='w1t')
        (nc.sync if c % 2 == 0 else nc.scalar).dma_start(w1t[:], w1_d[c])
        for m in range(4):
            mt = 4 * c + m
            ps = p256.tile([P, LL], F32, tag='p256')
            for k in range(KO):
                nc.tensor.matmul(ps[:], w1t[:, k, m * P:(m + 1) * P],
                                 ein16[:, k, :], start=(k == 0),
                                 stop=(k == KO - 1))
            nc.scalar.activation(h16[:, mt, :], ps[:], AF.Relu)
    for dt in range(KO):
        w2t = pools['w2p'].tile([P, MO, P], F16, tag='w2t')
        (nc.sync if dt % 2 == 0 else nc.scalar).dma_start(w2t[:], w2_d[dt])
        ps = p256.tile([P, LL], F32, tag='p256')
        for k in range(MO):
            nc.tensor.matmul(ps[:], w2t[:, k, :], h16[:, k, :],
                             start=(k == 0), stop=(k == MO - 1))
        nc.vector.tensor_tensor(Eres32[:, dt, :], Eres32[:, dt, :], ps[:],
                                mybir.AluOpType.add)


def _ln(nc, pools, Eres32, e16out, name):
    """In-place layernorm over features; writes fp16 copy to e16out."""
    p256 = pools['p256']
    ones = pools['ones']
    stat = pools['stat']

    e16pre = pools['lnp'].tile([P, KO, LL], F16, tag='e16pre')
    nc.vector.tensor_copy(e16pre[:], Eres32[:])
    sq16 = pools['lnp'].tile([P, KO, LL], F16, tag='sq16')
    nc.scalar.square(sq16[:], e16pre[:])
    pss = p256.tile([P, LL], F32, tag='p256')
    psq = p256.tile([P, LL], F32, tag='p256')
    for k in range(KO):
        nc.tensor.matmul(pss[:], ones[:, :], e16pre[:, k, :],
                         start=(k == 0), stop=(k == KO - 1))
    for k in range(KO):
        nc.tensor.matmul(psq[:], ones[:, :], sq16[:, k, :],
                         start=(k == 0), stop=(k == KO - 1))
    mean = stat.tile([P, LL], F32, tag='mean')
    nc.vector.tensor_scalar_mul(mean[:], pss[:], 1.0 / DE)
    varn = stat.tile([P, LL], F32, tag='varn')
    nc.vector.tensor_tensor(varn[:], pss[:], mean[:], mybir.AluOpType.mult)
    nc.vector.tensor_tensor(varn[:], psq[:], varn[:], mybir.AluOpType.subtract)
    std = stat.tile([P, LL], F32, tag='std')
    nc.scalar.activation(std[:], varn[:], AF.Sqrt,
                         bias=pools['eps128'], scale=1.0 / (DE - 1))
    inv = stat.tile([P, LL], F32, tag='inv')
    nc.vector.reciprocal_approx_fast(inv[:], std[:])
    ms = stat.tile([P, LL], F32, tag='ms')
    nc.vector.tensor_tensor(ms[:], mean[:], inv[:], mybir.AluOpType.mult)
    nc.vector.tensor_tensor(
        Eres32[:], Eres32[:],
        inv[:, None, :].to_broadcast((P, KO, LL)), mybir.AluOpType.mult)
    nc.vector.tensor_tensor(
        Eres32[:], Eres32[:],
        ms[:, None, :].to_broadcast((P, KO, LL)), mybir.AluOpType.subtract)
    nc.vector.tensor_copy(e16out[:], Eres32[:])
    tp = pools.get('tapfn')
    if tp:
        tp(f'{name}_out', Eres32)


def build_program(taps=()):
    taps = set(taps)
    nc = bacc.Bacc("TRN2", target_bir_lowering=False, debug=False,
                   num_devices=N_CORES)

    # ---- dram inputs ----
    din = {}

    def dram_in(nm, shape, dt=F16):
        din[nm] = nc.dram_tensor(nm, list(shape), dt, kind="ExternalInput")
        return din[nm]

    z0l32 = dram_in('z0_loc32', [P, KO, LL], F32)
    x0l32 = dram_in('x0_loc32', [P, KO, LL], F32)
    z0l16 = dram_in('z0_loc16', [P, KO, LL])
    x0l16 = dram_in('x0_loc16', [P, KO, LL])
    mask_self = dram_in('mask_self', [P, KT, LL])
    for pfx, nl in (('enc', LENC), ('dec', LDEC)):
        for w in ('wqT', 'wkT', 'wvT', 'woT'):
            dram_in(f'{pfx}_{w}', [nl, P, KO, DE])
        dram_in(f'{pfx}_w1T', [nl, 8, P, KO, 512])
        dram_in(f'{pfx}_w2T', [nl, 8, P, MO, P])
    wuT = dram_in('wuT', [NVCH, P, KO, VC])

    # output: [j, lt, ll, vc, vv] fp16
    outp = nc.dram_tensor('outp', [4, 4, P, NVCH, VC], F16,
                          kind="ExternalOutput")

    # internal dram for collectives (reused across attns; gpsimd-serialized)
    agin = nc.dram_tensor('agin', [P, 4096], F16)
    agout = nc.dram_tensor('agout', [2, P, 4096], F16)
    agf_in = [nc.dram_tensor(f'agf_in{i}', [P, 4, LL], F16) for i in range(2)]
    agf_out = [nc.dram_tensor(f'agf_out{i}', [N_CORES, P, 4, LL], F16,
                              addr_space='Shared') for i in range(2)]
    arin = nc.dram_tensor('arin', [P, 16], F32)
    arout = nc.dram_tensor('arout', [P, 16], F32, addr_space="Shared")

    import contextlib
    with tile.TileContext(nc) as tc, contextlib.ExitStack() as octx:
        const = octx.enter_context(tc.tile_pool(name='const', bufs=1))
        ones = const.tile([P, P], F16)
        nc.vector.memset(ones[:], 1.0)
        eps128 = const.tile([P, 1], F32)
        nc.vector.memset(eps128[:], EPS)
        msk = const.tile([P, KT, LL], F16)
        nc.sync.dma_start(msk[:], mask_self[:])

        # ================= layer phase =================
        with contextlib.ExitStack() as ctx:
            stream = ctx.enter_context(tc.tile_pool(name='stream', bufs=1))
            att = ctx.enter_context(tc.tile_pool(name='att', bufs=1))
            mlpp = ctx.enter_context(tc.tile_pool(name='mlpp', bufs=1))
            lnp = ctx.enter_context(tc.tile_pool(name='lnp', bufs=1))
            stat = ctx.enter_context(tc.tile_pool(name='stat', bufs=1))
            wqp = ctx.enter_context(tc.tile_pool(name='wqp', bufs=1))
            wkp = ctx.enter_context(tc.tile_pool(name='wkp', bufs=1))
            wvp = ctx.enter_context(tc.tile_pool(name='wvp', bufs=1))
            wop = ctx.enter_context(tc.tile_pool(name='wop', bufs=1))
            w1p = ctx.enter_context(tc.tile_pool(name='w1p', bufs=2))
            w2p = ctx.enter_context(tc.tile_pool(name='w2p', bufs=2))
            p256 = ctx.enter_context(tc.tile_pool(name='p256', bufs=2,
                                                  space='PSUM'))
            p512 = ctx.enter_context(tc.tile_pool(name='p512', bufs=4,
                                                  space='PSUM'))

            pools = dict(att=att, mlp=mlpp, lnp=lnp, p256=p256, p512=p512,
                         stat=stat, ones=ones, eps128=eps128[:],
                         wqp=wqp, wkp=wkp, wvp=wvp, wop=wop, w1p=w1p, w2p=w2p)

            def tapfn(nm, t):
                if nm not in taps:
                    return
                d = nc.dram_tensor('tap_' + nm, list(t.shape),
                                   t.dtype, kind="ExternalOutput")
                nc.sync.dma_start(d[:], t[:])
            pools['tapfn'] = tapfn

            # ======== encoder ========
            Eres = stream.tile([P, KO, LL], F32, tag='res')
            nc.sync.dma_start(Eres[:], z0l32[:])
            eloc = stream.tile([P, KO, LL], F16, tag='loc_a')
            nc.sync.dma_start(eloc[:], z0l16[:])

            for l in range(LENC):
                W = _load_attn_w(nc, pools, din['enc_wqT'][l],
                                 din['enc_wkT'][l], din['enc_wvT'][l],
                                 din['enc_woT'][l])
                kfull = att.tile([P, KO, L], F16, tag='kfull')
                vt16 = att.tile([P, KT, H * DA], F16, tag='vt16')
                _kv_proj_ag(nc, pools, W, eloc, agin, agout, kfull, vt16)
                _attn_core(nc, pools, Eres, eloc, W, kfull, vt16, None,
                           f'e{l}a')
                eloc = stream.tile([P, KO, LL], F16, tag='loc_b')
                _ln(nc, pools, Eres, eloc, f'e{l}ln1')
                _mlp(nc, pools, Eres, eloc, din['enc_w1T'][l],
                     din['enc_w2T'][l], f'e{l}m')
                eloc = stream.tile([P, KO, LL], F16, tag='loc_a')
                _ln(nc, pools, Eres, eloc, f'e{l}ln2')

            Zloc = stream.tile([P, KO, LL], F16, tag='zloc')
            nc.vector.tensor_copy(Zloc[:], eloc[:])

            # ======== decoder ========
            Eres = stream.tile([P, KO, LL], F32, tag='res')
            nc.sync.dma_start(Eres[:], x0l32[:])
            eloc = stream.tile([P, KO, LL], F16, tag='loc_a')
            nc.sync.dma_start(eloc[:], x0l16[:])

            for l in range(LDEC):
                W = _load_attn_w(nc, pools, din['dec_wqT'][l],
                                 din['dec_wkT'][l], din['dec_wvT'][l],
                                 din['dec_woT'][l])
                # self K/V + AG
                kfull_s = att.tile([P, KO, L], F16, tag='kfull')
                vt16_s = att.tile([P, KT, H * DA], F16, tag='vt16')
                _kv_proj_ag(nc, pools, W, eloc, agin, agout, kfull_s, vt16_s)
                # cross K/V + AG (overlaps self AG; weights shared)
                kfull_c = att.tile([P, KO, L], F16, tag='kfull_c')
                vt16_c = att.tile([P, KT, H * DA], F16, tag='vt16_c')
                _kv_proj_ag(nc, pools, W, Zloc, agin, agout, kfull_c, vt16_c)
                # self attention (causal)
                _attn_core(nc, pools, Eres, eloc, W, kfull_s, vt16_s, msk,
                           f'd{l}s')
                eloc = stream.tile([P, KO, LL], F16, tag='loc_b')
                _ln(nc, pools, Eres, eloc, f'd{l}ln1')
                # cross attention
                _attn_core(nc, pools, Eres, eloc, W, kfull_c, vt16_c, None,
                           f'd{l}c')
                eloc = stream.tile([P, KO, LL], F16, tag='loc_b')
                _ln(nc, pools, Eres, eloc, f'd{l}ln2')
                _mlp(nc, pools, Eres, eloc, din['dec_w1T'][l],
                     din['dec_w2T'][l], f'd{l}m')
                eloc = stream.tile([P, KO, LL], F16, tag='loc_a')
                _ln(nc, pools, Eres, eloc, f'd{l}ln3')

            # final activations for the 8-core gather, in 2 chunks
            for i in range(2):
                nc.gpsimd.dma_start(agf_in[i][:], eloc[:, 4 * i:4 * i + 4, :])
                nc.gpsimd.collective_compute(
                    "AllGather", mybir.AluOpType.bypass,
                    ins=[agf_in[i][:]], outs=[agf_out[i][:]],
                    replica_groups=ALL_GROUP)

        # ================= unembed phase (transposed) =================
        with contextlib.ExitStack() as ctx:
            usb = ctx.enter_context(tc.tile_pool(name='usb', bufs=1))
            ures_p = ctx.enter_context(tc.tile_pool(name='ures_p', bufs=3))
            wup = ctx.enter_context(tc.tile_pool(name='wup', bufs=3))
            u512 = ctx.enter_context(tc.tile_pool(name='u512', bufs=6,
                                                  space='PSUM'))

            # XF: [128(d within tile), ko, j, l]  fp16
            XF = usb.tile([P, KO, 4, L], F16, tag='XF')
            for i in range(2):
                for r in range(N_CORES):
                    nc.sync.dma_start(
                        XF[:, 4 * i:4 * i + 4, r // 2,
                           (r % 2) * LL:(r % 2) * LL + LL],
                        agf_out[i][r])
            if 'xf' in taps:
                d = nc.dram_tensor('tap_xf', [P, KO, 4, L], F16,
                                   kind="ExternalOutput")
                nc.sync.dma_start(d[:], XF[:])

            expu = usb.tile([P, NVCH, 16, VC], F16, tag='expu')
            dacc = usb.tile([P, NVCH, 16], F32, tag='dacc')
            for vc in range(NVCH):
                wut = wup.tile([P, KO, VC], F16, tag='wut')
                (nc.sync if vc % 2 == 0 else nc.scalar).dma_start(
                    wut[:], wuT[vc])
                for j in range(4):
                    for lt in range(4):
                        jlt = 4 * j + lt
                        ps = u512.tile([P, VC], F32, tag='u512')
                        for k in range(KO):
                            nc.tensor.matmul(
                                ps[:], XF[:, k, j, lt * P:(lt + 1) * P],
                                wut[:, k, :], start=(k == 0),
                                stop=(k == KO - 1))
                        nc.scalar.activation(
                            expu[:, vc, jlt, :], ps[:], AF.Exp,
                            accum_out=dacc[:, vc, jlt:jlt + 1])
            # reduce deno over vocab chunks, AllReduce over cores
            d4 = usb.tile([P, 4, 16], F32, tag='d4')
            nc.vector.tensor_tensor(d4[:], dacc[:, 0:4, :], dacc[:, 4:8, :],
                                    mybir.AluOpType.add)
            d2 = usb.tile([P, 2, 16], F32, tag='d2')
            nc.vector.tensor_tensor(d2[:], d4[:, 0:2, :], d4[:, 2:4, :],
                                    mybir.AluOpType.add)
            deno = usb.tile([P, 16], F32, tag='deno')
            nc.vector.tensor_tensor(deno[:], d2[:, 0, :], d2[:, 1, :],
                                    mybir.AluOpType.add)
            nc.gpsimd.dma_start(arin[:], deno[:])
            nc.gpsimd.collective_compute(
                "AllReduce", mybir.AluOpType.add,
                ins=[arin[:]], outs=[arout[:]], replica_groups=ALL_GROUP)
            denof = usb.tile([P, 16], F32, tag='denof')
            nc.sync.dma_start(denof[:], arout[:])
            binv = usb.tile([P, 16], F32, tag='binv')
            nc.vector.reciprocal_approx_fast(binv[:], denof[:])
            if 'deno' in taps:
                d = nc.dram_tensor('tap_deno', [P, 16], F32,
                                   kind="ExternalOutput")
                nc.sync.dma_start(d[:], denof[:])

            # normalize + store (split across engines/queues)
            dmae = [nc.sync, nc.gpsimd, nc.scalar]
            for vc in range(NVCH):
                for j in range(4):
                    for lt in range(4):
                        jlt = 4 * j + lt
                        res = ures_p.tile([P, VC], F16, tag='ures')
                        i = (vc * 16 + jlt) % 3
                        if i == 0:
                            nc.vector.tensor_tensor(
                                res[:], expu[:, vc, jlt, :],
                                binv[:, jlt:jlt + 1].to_broadcast((P, VC)),
                                mybir.AluOpType.mult)
                        elif i == 1:
                            nc.scalar.activation(
                                res[:], expu[:, vc, jlt, :], AF.Copy,
                                scale=binv[:, jlt:jlt + 1])
                        else:
                            nc.gpsimd.tensor_tensor(
                                res[:], expu[:, vc, jlt, :],
                                binv[:, jlt:jlt + 1].to_broadcast((P, VC)),
                                mybir.AluOpType.mult)
                        dmae[(vc * 16 + jlt) % 3].dma_start(
                            outp[j, lt, :, vc, :], res[:])

    nc.compile()
    return nc


# ----------------------------------------------------------------------------
# host-side prep
# ----------------------------------------------------------------------------

def _to_kimaj(a):
    """[K, M] -> [128, K//128, M] with K = ko*128 + ki."""
    K, M = a.shape
    return np.ascontiguousarray(
        a.reshape(K // P, P, M).transpose(1, 0, 2))


def prep_inputs(inputs):
    f = lambda k: np.asarray(inputs[k], dtype=np.float32)
    We, Wp, Wu = f('We'), f('Wp'), f('Wu')
    x = np.asarray(inputs['x']).astype(np.int64)
    z = np.asarray(inputs['z']).astype(np.int64)

    shared = {}
    for pfx, nl in (('enc', LENC), ('dec', LDEC)):
        Wq, Wk, Wv = f(pfx + '_Wq'), f(pfx + '_Wk'), f(pfx + '_Wv')
        Wo, W1, W2 = f(pfx + '_Wo'), f(pfx + '_W1'), f(pfx + '_W2')
        wq, wk, wv, wo, w1, w2 = [], [], [], [], [], []
        for l in range(nl):
            qa = Wq[l].transpose(2, 0, 1).reshape(DE, H * DA) * (DA ** -0.5)
            ka = Wk[l].transpose(2, 0, 1).reshape(DE, H * DA)
            va = Wv[l].transpose(2, 0, 1).reshape(DE, H * DA)
            wq.append(_to_kimaj(qa))
            wk.append(_to_kimaj(ka))
            wv.append(_to_kimaj(va))
            wo.append(_to_kimaj(Wo[l].T))
            w1k = _to_kimaj(W1[l].T)          # [128, 8, 4096]
            w1.append(np.ascontiguousarray(
                w1k.reshape(P, KO, 8, 512).transpose(2, 0, 1, 3)))
            w2k = _to_kimaj(W2[l].T)          # [128, 32, 1024]
            w2.append(np.ascontiguousarray(
                w2k.reshape(P, MO, 8, P).transpose(2, 0, 1, 3)))
        shared[f'{pfx}_wqT'] = np.stack(wq).astype(np.float16)
        shared[f'{pfx}_wkT'] = np.stack(wk).astype(np.float16)
        shared[f'{pfx}_wvT'] = np.stack(wv).astype(np.float16)
        shared[f'{pfx}_woT'] = np.stack(wo).astype(np.float16)
        shared[f'{pfx}_w1T'] = np.stack(w1).astype(np.float16)
        shared[f'{pfx}_w2T'] = np.stack(w2).astype(np.float16)

    pos = Wp[:L]  # [512, 1024]
    in_maps = []
    for c in range(N_CORES):
        b, h = c // 2, c % 2
        m = dict(shared)
        for nm, tok in (('z0', z[b]), ('x0', x[b])):
            E0 = (We[tok] + pos).T.astype(np.float32)      # [1024, 512]
            E0k = E0.reshape(KO, P, L)                     # [ko, ki, p]
            loc = E0k[:, :, h * LL:(h + 1) * LL].transpose(1, 0, 2)
            m[nm + '_loc32'] = np.ascontiguousarray(loc)
            m[nm + '_loc16'] = np.ascontiguousarray(loc).astype(np.float16)
        kglob = np.arange(L)[:, None]
        qglob = (h * LL + np.arange(LL))[None, :]
        msk = (kglob <= qglob).astype(np.float16)          # [512, 256]
        m['mask_self'] = np.ascontiguousarray(
            msk.reshape(KT, P, LL).transpose(1, 0, 2))
        wus = Wu[c * NVC:(c + 1) * NVC].T                  # [1024, 4000]
        wuk = _to_kimaj(wus)                               # [128, 8, 4000]
        m['wuT'] = np.ascontiguousarray(
            wuk.reshape(P, KO, NVCH, VC).transpose(2, 0, 1, 3)
        ).astype(np.float16)
        in_maps.append(m)
    return in_maps


def assemble(results):
    """results: per-core dicts with 'outp' [4, 4, 128, NVCH, VC] fp16."""
    out = np.empty((4, NV, L), dtype=np.float32)
    for c, r in enumerate(results):
        o = np.asarray(r['outp'], dtype=np.float32)  # [j, lt, ll, vc, vv]
        o = o.transpose(0, 3, 4, 1, 2).reshape(4, NVC, L)
        out[:, c * NVC:(c + 1) * NVC, :] = o
    return out


def run(inputs, trace=False, taps=(), trace_kwargs=None):
    key = ('prog', tuple(sorted(taps)))
    if key not in _CACHE:
        _CACHE[key] = build_program(taps=taps)
    nc = _CACHE[key]
    in_maps = prep_inputs(inputs)
    res = run_bass_kernel_spmd(nc, in_maps, list(range(N_CORES)),
                               trace=trace, **(trace_kwargs or {}))
    return res


def kernel(**inputs):
    res = run(inputs, trace=False)
    return assemble(res.results)


# revision 19
# speedup vs baseline: 1.1604x; 1.0035x over previous
"""Trainium2 Bass kernel for nn_EDTransformer (encoder-decoder transformer).

Sharding: 8 cores = 4 batch items x 2 sequence halves.
 - Each core owns (item b, half h): computes Q/scores/AV/Wo/MLP/LN for its
   256 local positions; K/V computed for the LOCAL half only and completed
   via a 2-core AllGather of K/V per attention block.
 - Decoder self+cross attention share one weight load per layer.
 - Unembedding sharded over vocab (4000 rows/core, 8 chunks of 500),
   computed TRANSPOSED (positions on partitions) so the softmax denominator
   comes from the Act engine accumulator and the normalize is a
   per-partition scale; denominator summed via one 8-core AllReduce.
 - Weights pre-tiled host-side so every DMA reads contiguous >=2KB runs
   per partition; weight loads spread across sync/scalar queues.
Dtypes: fp16 matmul operands, fp32 PSUM, fp32 residual + LN stats,
 fp16 output (cast to fp32 on host).
"""
import os
import sys

sys.path.insert(0, '/opt/trn_rl_repo')
import numpy as np

import concourse.bacc as bacc
import concourse.tile as tile
import concourse.mybir as mybir
from concourse.bass_utils import run_bass_kernel_spmd

DT = mybir.dt
F16 = DT.float16
F32 = DT.float32
AF = mybir.ActivationFunctionType

N_CORES = 8
P = 128
DE = 1024           # model dim (8 ptiles)
KO = DE // P        # 8
DMLP = 4096         # mlp dim
MO = DMLP // P      # 32
H = 16              # heads
DA = 64             # attn dim per head
L = 512             # sequence length
LL = 256            # local positions per core
KT = L // P         # 4 key tiles
NV = 32000
NVC = NV // N_CORES  # 4000 vocab rows per core
VC = 500            # vocab chunk (8 chunks of 500)
NVCH = NVC // VC    # 8
LENC = 2
LDEC = 2
EPS = 1e-5

PAIR_GROUPS = [[0, 1], [2, 3], [4, 5], [6, 7]]
ALL_GROUP = [list(range(N_CORES))]

_CACHE = {}


# ----------------------------------------------------------------------------
# device program
# ----------------------------------------------------------------------------

def _kv_proj_ag(nc, pools, W, kvin16, agin, agout, kfull, vt16):
    """Project K/V from local stream and pair-AllGather to full length.

    kvin16: [128, KO, LL] local stream.
    kfull : [128, KO, L]  (partitions = 2h x 64a rows)
    vt16  : [128, KT, H*DA] (partitions = key positions)
    """
    p256 = pools['p256']
    p512 = pools['p512']
    kloc = pools['att'].tile([P, KO, LL], F16, tag='kloc')
    for pr in range(KO):
        ps = p256.tile([P, LL], F32, tag='p256')
        for k in range(KO):
            nc.tensor.matmul(ps[:], W['wk'][:, k, pr * P:(pr + 1) * P],
                             kvin16[:, k, :], start=(k == 0), stop=(k == KO - 1))
        nc.vector.tensor_copy(kloc[:, pr, :], ps[:])
    vloc = pools['att'].tile([P, 2, H * DA], F16, tag='vloc')
    for lc in range(2):
        for nch in range(2):
            ps = p512.tile([P, 512], F32, tag='p512', bufs=2)
            for k in range(KO):
                nc.tensor.matmul(ps[:], kvin16[:, k, lc * P:(lc + 1) * P],
                                 W['wv'][:, k, nch * 512:(nch + 1) * 512],
                                 start=(k == 0), stop=(k == KO - 1))
            nc.vector.tensor_copy(vloc[:, lc, nch * 512:(nch + 1) * 512], ps[:])
    # stage k (2048) + v (2048) into one dram buffer, AllGather over the pair
    nc.gpsimd.dma_start(
        agin[:, 0:2048].rearrange('p (a b) -> p a b', a=KO), kloc[:])
    nc.gpsimd.dma_start(
        agin[:, 2048:4096].rearrange('p (a b) -> p a b', a=2), vloc[:])
    nc.gpsimd.collective_compute(
        "AllGather", mybir.AluOpType.bypass,
        ins=[agin[:]], outs=[agout[:]], replica_groups=PAIR_GROUPS)
    for r in range(2):
        nc.gpsimd.dma_start(
            kfull[:, :, r * LL:(r + 1) * LL],
            agout[r, :, 0:2048].rearrange('p (a b) -> p a b', a=KO))
        nc.gpsimd.dma_start(
            vt16[:, r * 2:(r + 1) * 2, :],
            agout[r, :, 2048:4096].rearrange('p (a b) -> p a b', a=2))


def _attn_core(nc, pools, Eres32, qin16, W, kfull, vt16, mask, name):
    """Q projection, scores/softmax/AV per head-pair, Wo accumulate."""
    p256 = pools['p256']
    p512 = pools['p512']
    ones = pools['ones']
    sb = pools['att']

    q16 = sb.tile([P, KO, LL], F16, tag='q16')
    for pr in range(KO):
        ps = p256.tile([P, LL], F32, tag='p256')
        for k in range(KO):
            nc.tensor.matmul(ps[:], W['wq'][:, k, pr * P:(pr + 1) * P],
                             qin16[:, k, :], start=(k == 0), stop=(k == KO - 1))
        nc.vector.tensor_copy(q16[:, pr, :], ps[:])

    y16 = sb.tile([P, KO, LL], F16, tag='y16')
    for pr in range(KO):
        hA, hB = 2 * pr, 2 * pr + 1
        # scores -> exp, 2 heads x 4 kt; psum pairs give [128, 512] exps
        exp16 = sb.tile([P, 2, KT, LL], F16, tag='exp16', bufs=2)
        for hh in range(2):
            h = hA + hh
            hp = (h % 2) * DA
            for kp in range(2):
                ps = p512.tile([P, 2, LL], F32, tag='psc', bufs=2)
                for ki in range(2):
                    kt = 2 * kp + ki
                    nc.tensor.matmul(
                        ps[:, ki, :],
                        kfull[hp:hp + DA, pr, kt * P:(kt + 1) * P],
                        q16[hp:hp + DA, pr, :], start=True, stop=True)
                nc.scalar.activation(exp16[:, hh, 2 * kp:2 * kp + 2, :],
                                     ps[:], AF.Exp)
        if mask is not None:
            nc.vector.tensor_tensor(
                exp16[:], exp16[:],
                mask[:, None, :, :].to_broadcast((P, 2, KT, LL)),
                mybir.AluOpType.mult)
        # denominators (replicated over partitions via ones matmul)
        pd = p512.tile([P, 2, LL], F32, tag='pd', bufs=2)
        for kt in range(KT):
            nc.tensor.matmul(pd[:], ones[:, :], exp16[:, :, kt, :],
                             start=(kt == 0), stop=(kt == KT - 1))
        ysc = pools['stat'].tile([P, 2, LL], F32, tag='ysc', bufs=2)
        nc.vector.reciprocal_approx_fast(ysc[:], pd[:])
        # AV (2-head column packing)
        ps = p256.tile([P, LL], F32, tag='p256')
        for kt in range(KT):
            nc.tensor.matmul(ps[:DA, :], vt16[:, kt, hA * DA:(hA + 1) * DA],
                             exp16[:, 0, kt, :], start=(kt == 0),
                             stop=(kt == KT - 1), tile_position=(0, 0))
            nc.tensor.matmul(ps[DA:, :], vt16[:, kt, hB * DA:(hB + 1) * DA],
                             exp16[:, 1, kt, :], start=(kt == 0),
                             stop=(kt == KT - 1), tile_position=(0, DA))
        nc.vector.tensor_tensor(y16[:DA, pr, :], ps[:DA, :], ysc[:DA, 0, :],
                                mybir.AluOpType.mult)
        nc.vector.tensor_tensor(y16[DA:, pr, :], ps[DA:, :], ysc[DA:, 1, :],
                                mybir.AluOpType.mult)

    # Wo -> accumulate into residual
    for dt in range(KO):
        ps = p256.tile([P, LL], F32, tag='p256')
        for k in range(KO):
            nc.tensor.matmul(ps[:], W['wo'][:, k, dt * P:(dt + 1) * P],
                             y16[:, k, :], start=(k == 0), stop=(k == KO - 1))
        nc.vector.tensor_tensor(Eres32[:, dt, :], Eres32[:, dt, :], ps[:],
                                mybir.AluOpType.add)
    tp = pools.get('tapfn')
    if tp:
        tp(f'{name}_q', q16)
        tp(f'{name}_y', y16)


def _load_attn_w(nc, pools, wq_d, wk_d, wv_d, wo_d):
    wq = pools['wqp'].tile([P, KO, DE], F16, tag='wq')
    nc.sync.dma_start(wq[:], wq_d[:])
    wk = pools['wkp'].tile([P, KO, DE], F16, tag='wk')
    nc.scalar.dma_start(wk[:], wk_d[:])
    wv = pools['wvp'].tile([P, KO, DE], F16, tag='wv')
    nc.sync.dma_start(wv[:], wv_d[:])
    wo = pools['wop'].tile([P, KO, DE], F16, tag='wo')
    nc.scalar.dma_start(wo[:], wo_d[:])
    return {'wq': wq, 'wk': wk, 'wv': wv, 'wo': wo}


def _mlp(nc, pools, Eres32, ein16, w1_d, w2_d, name):
    p256 = pools['p256']
    h16 = pools['mlp'].tile([P, MO, LL], F16, tag='h16')
    for c in range(8):
        w1t = pools['w1p'].tile([P, KO, 512], F16, tag='w1t')
        (nc.sync if c % 2 == 0 else nc.scalar).dma_start(w1t[:], w1_d[c])
        for m in range(4):
            mt = 4 * c + m
            ps = p256.tile([P, LL], F32, tag='p256')
            for k in range(KO):
                nc.tensor.matmul(ps[:], w1t[:, k, m * P:(m + 1) * P],
                                 ein16[:, k, :], start=(k == 0),
                                 stop=(k == KO - 1))
            nc.scalar.activation(h16[:, mt, :], ps[:], AF.Relu)
    for dt in range(KO):
        w2t = pools['w2p'].tile([P, MO, P], F16, tag='w2t')
        (nc.sync if dt % 2 == 0 else nc.scalar).dma_start(w2t[:], w2_d[dt])
        ps = p256.tile([P, LL], F32, tag='p256')
        for k in range(MO):
            nc.tensor.matmul(ps[:], w2t[:, k, :], h16[:, k, :],
                             start=(k == 0), stop=(k == MO - 1))
        nc.vector.tensor_tensor(Eres32[:, dt, :], Eres32[:, dt, :], ps[:],
                                mybir.AluOpType.add)


def _ln(nc, pools, Eres32, e16out, name):
    """In-place layernorm over features; writes fp16 copy to e16out."""
    p256 = pools['p256']
    ones = pools['ones']
    stat = pools['stat']

    e16pre = pools['lnp'].tile([P, KO, LL], F16, tag='e16pre')
    nc.vector.tensor_copy(e16pre[:], Eres32[:])
    sq16 = pools['lnp'].tile([P, KO, LL], F16, tag='sq16')
    nc.scalar.square(sq16[:], e16pre[:])
    pss = p256.tile([P, LL], F32, tag='p256')
    psq = p256.tile([P, LL], F32, tag='p256')
    for k in range(KO):
        nc.tensor.matmul(pss[:], ones[:, :], e16pre[:, k, :],
                         start=(k == 0), stop=(k == KO - 1))
    for k in range(KO):
        nc.tensor.matmul(psq[:], ones[:, :], sq16[:, k, :],
                         start=(k == 0), stop=(k == KO - 1))
    mean = stat.tile([P, LL], F32, tag='mean')
    nc.vector.tensor_scalar_mul(mean[:], pss[:], 1.0 / DE)
    varn = stat.tile([P, LL], F32, tag='varn')
    nc.vector.tensor_tensor(varn[:], pss[:], mean[:], mybir.AluOpType.mult)
    nc.vector.tensor_tensor(varn[:], psq[:], varn[:], mybir.AluOpType.subtract)
    std = stat.tile([P, LL], F32, tag='std')
    nc.scalar.activation(std[:], varn[:], AF.Sqrt,
                         bias=pools['eps128'], scale=1.0 / (DE - 1))
    inv = stat.tile([P, LL], F32, tag='inv')
    nc.vector.reciprocal_approx_fast(inv[:], std[:])
    ms = stat.tile([P, LL], F32, tag='ms')
    nc.vector.tensor_tensor(ms[:], mean[:], inv[:], mybir.AluOpType.mult)
    nc.vector.tensor_tensor(
        Eres32[:], Eres32[:],
        inv[:, None, :].to_broadcast((P, KO, LL)), mybir.AluOpType.mult)
    nc.vector.tensor_tensor(
        Eres32[:], Eres32[:],
        ms[:, None, :].to_broadcast((P, KO, LL)), mybir.AluOpType.subtract)
    nc.vector.tensor_copy(e16out[:], Eres32[:])
    tp = pools.get('tapfn')
    if tp:
        tp(f'{name}_out', Eres32)


def build_program(taps=()):
    taps = set(taps)
    nc = bacc.Bacc("TRN2", target_bir_lowering=False, debug=False,
                   num_devices=N_CORES)

    # ---- dram inputs ----
    din = {}

    def dram_in(nm, shape, dt=F16):
        din[nm] = nc.dram_tensor(nm, list(shape), dt, kind="ExternalInput")
        return din[nm]

    z0l32 = dram_in('z0_loc32', [P, KO, LL], F32)
    x0l32 = dram_in('x0_loc32', [P, KO, LL], F32)
    z0l16 = dram_in('z0_loc16', [P, KO, LL])
    x0l16 = dram_in('x0_loc16', [P, KO, LL])
    mask_self = dram_in('mask_self', [P, KT, LL])
    for pfx, nl in (('enc', LENC), ('dec', LDEC)):
        for w in ('wqT', 'wkT', 'wvT', 'woT'):
            dram_in(f'{pfx}_{w}', [nl, P, KO, DE])
        dram_in(f'{pfx}_w1T', [nl, 8, P, KO, 512])
        dram_in(f'{pfx}_w2T', [nl, 8, P, MO, P])
    wuT = dram_in('wuT', [NVCH, P, KO, VC])

    # output: [vc, j, lt, ll, vv] fp16 (each store is one contiguous block)
    outp = nc.dram_tensor('outp', [NVCH, 4, 4, P, VC], F16,
                          kind="ExternalOutput")

    # internal dram for collectives (reused across attns; gpsimd-serialized)
    agin = nc.dram_tensor('agin', [P, 4096], F16)
    agout = nc.dram_tensor('agout', [2, P, 4096], F16)
    agf_in = [nc.dram_tensor(f'agf_in{i}', [P, 4, LL], F16) for i in range(2)]
    agf_out = [nc.dram_tensor(f'agf_out{i}', [N_CORES, P, 4, LL], F16,
                              addr_space='Shared') for i in range(2)]
    arin = nc.dram_tensor('arin', [P, 16], F32)
    arout = nc.dram_tensor('arout', [P, 16], F32, addr_space="Shared")
    # tiny warm-up buffers: trigger CC channel init at t=0
    wu_in = nc.dram_tensor('wu_in', [P, 1], F16)
    wu_p_out = nc.dram_tensor('wu_p_out', [2, P, 1], F16)
    wu_a_out = nc.dram_tensor('wu_a_out', [N_CORES, P, 1], F16,
                              addr_space='Shared')
    wu_r_in = nc.dram_tensor('wu_r_in', [P, 1], F32)
    wu_r_out = nc.dram_tensor('wu_r_out', [P, 1], F32, addr_space='Shared')

    import contextlib
    with tile.TileContext(nc) as tc, contextlib.ExitStack() as octx:
        const = octx.enter_context(tc.tile_pool(name='const', bufs=1))
        ones = const.tile([P, P], F16)
        nc.vector.memset(ones[:], 1.0)
        eps128 = const.tile([P, 1], F32)
        nc.vector.memset(eps128[:], EPS)
        msk = const.tile([P, KT, LL], F16)
        nc.sync.dma_start(msk[:], mask_self[:])

        # warm up the CC channels (pair + all-group) so real collectives
        # don't pay channel-init latency mid-kernel
        wtile = const.tile([P, 1], F16)
        nc.vector.memset(wtile[:], 0.0)
        nc.gpsimd.dma_start(wu_in[:], wtile[:])
        wtile32 = const.tile([P, 1], F32)
        nc.vector.memset(wtile32[:], 0.0)
        nc.gpsimd.dma_start(wu_r_in[:], wtile32[:])
        nc.gpsimd.collective_compute(
            "AllGather", mybir.AluOpType.bypass,
            ins=[wu_in[:]], outs=[wu_p_out[:]], replica_groups=PAIR_GROUPS)
        nc.gpsimd.collective_compute(
            "AllGather", mybir.AluOpType.bypass,
            ins=[wu_in[:]], outs=[wu_a_out[:]], replica_groups=ALL_GROUP)
        nc.gpsimd.collective_compute(
            "AllReduce", mybir.AluOpType.add,
            ins=[wu_r_in[:]], outs=[wu_r_out[:]], replica_groups=ALL_GROUP)

        # ================= layer phase =================
        with contextlib.ExitStack() as ctx:
            stream = ctx.enter_context(tc.tile_pool(name='stream', bufs=1))
            att = ctx.enter_context(tc.tile_pool(name='att', bufs=1))
            mlpp = ctx.enter_context(tc.tile_pool(name='mlpp', bufs=1))
            lnp = ctx.enter_context(tc.tile_pool(name='lnp', bufs=1))
            stat = ctx.enter_context(tc.tile_pool(name='stat', bufs=1))
            wqp = ctx.enter_context(tc.tile_pool(name='wqp', bufs=1))
            wkp = ctx.enter_context(tc.tile_pool(name='wkp', bufs=1))
            wvp = ctx.enter_context(tc.tile_pool(name='wvp', bufs=1))
            wop = ctx.enter_context(tc.tile_pool(name='wop', bufs=1))
            w1p = ctx.enter_context(tc.tile_pool(name='w1p', bufs=2))
            w2p = ctx.enter_context(tc.tile_pool(name='w2p', bufs=2))
            p256 = ctx.enter_context(tc.tile_pool(name='p256', bufs=2,
                                                  space='PSUM'))
            p512 = ctx.enter_context(tc.tile_pool(name='p512', bufs=4,
                                                  space='PSUM'))

            pools = dict(att=att, mlp=mlpp, lnp=lnp, p256=p256, p512=p512,
                         stat=stat, ones=ones, eps128=eps128[:],
                         wqp=wqp, wkp=wkp, wvp=wvp, wop=wop, w1p=w1p, w2p=w2p)

            def tapfn(nm, t):
                if nm not in taps:
                    return
                d = nc.dram_tensor('tap_' + nm, list(t.shape),
                                   t.dtype, kind="ExternalOutput")
                nc.sync.dma_start(d[:], t[:])
            pools['tapfn'] = tapfn

            # ======== encoder ========
            Eres = stream.tile([P, KO, LL], F32, tag='res')
            nc.sync.dma_start(Eres[:], z0l32[:])
            eloc = stream.tile([P, KO, LL], F16, tag='loc_a')
            nc.sync.dma_start(eloc[:], z0l16[:])

            for l in range(LENC):
                W = _load_attn_w(nc, pools, din['enc_wqT'][l],
                                 din['enc_wkT'][l], din['enc_wvT'][l],
                                 din['enc_woT'][l])
                kfull = att.tile([P, KO, L], F16, tag='kfull')
                vt16 = att.tile([P, KT, H * DA], F16, tag='vt16')
                _kv_proj_ag(nc, pools, W, eloc, agin, agout, kfull, vt16)
                _attn_core(nc, pools, Eres, eloc, W, kfull, vt16, None,
                           f'e{l}a')
                eloc = stream.tile([P, KO, LL], F16, tag='loc_b')
                _ln(nc, pools, Eres, eloc, f'e{l}ln1')
                _mlp(nc, pools, Eres, eloc, din['enc_w1T'][l],
                     din['enc_w2T'][l], f'e{l}m')
                eloc = stream.tile([P, KO, LL], F16, tag='loc_a')
                _ln(nc, pools, Eres, eloc, f'e{l}ln2')

            Zloc = stream.tile([P, KO, LL], F16, tag='zloc')
            nc.vector.tensor_copy(Zloc[:], eloc[:])

            # ======== decoder ========
            Eres = stream.tile([P, KO, LL], F32, tag='res')
            nc.sync.dma_start(Eres[:], x0l32[:])
            eloc = stream.tile([P, KO, LL], F16, tag='loc_a')
            nc.sync.dma_start(eloc[:], x0l16[:])

            for l in range(LDEC):
                W = _load_attn_w(nc, pools, din['dec_wqT'][l],
                                 din['dec_wkT'][l], din['dec_wvT'][l],
                                 din['dec_woT'][l])
                # self K/V + AG
                kfull_s = att.tile([P, KO, L], F16, tag='kfull')
                vt16_s = att.tile([P, KT, H * DA], F16, tag='vt16')
                _kv_proj_ag(nc, pools, W, eloc, agin, agout, kfull_s, vt16_s)
                # cross K/V + AG (overlaps self AG; weights shared)
                kfull_c = att.tile([P, KO, L], F16, tag='kfull_c')
                vt16_c = att.tile([P, KT, H * DA], F16, tag='vt16_c')
                _kv_proj_ag(nc, pools, W, Zloc, agin, agout, kfull_c, vt16_c)
                # self attention (causal)
                _attn_core(nc, pools, Eres, eloc, W, kfull_s, vt16_s, msk,
                           f'd{l}s')
                eloc = stream.tile([P, KO, LL], F16, tag='loc_b')
                _ln(nc, pools, Eres, eloc, f'd{l}ln1')
                # cross attention
                _attn_core(nc, pools, Eres, eloc, W, kfull_c, vt16_c, None,
                           f'd{l}c')
                eloc = stream.tile([P, KO, LL], F16, tag='loc_b')
                _ln(nc, pools, Eres, eloc, f'd{l}ln2')
                _mlp(nc, pools, Eres, eloc, din['dec_w1T'][l],
                     din['dec_w2T'][l], f'd{l}m')
                eloc = stream.tile([P, KO, LL], F16, tag='loc_a')
                _ln(nc, pools, Eres, eloc, f'd{l}ln3')

            # final activations for the 8-core gather, in 2 chunks
            for i in range(2):
                nc.gpsimd.dma_start(agf_in[i][:], eloc[:, 4 * i:4 * i + 4, :])
                nc.gpsimd.collective_compute(
                    "AllGather", mybir.AluOpType.bypass,
                    ins=[agf_in[i][:]], outs=[agf_out[i][:]],
                    replica_groups=ALL_GROUP)

        # ================= unembed phase (transposed) =================
        with contextlib.ExitStack() as ctx:
            usb = ctx.enter_context(tc.tile_pool(name='usb', bufs=1))
            ures_p = ctx.enter_context(tc.tile_pool(name='ures_p', bufs=3))
            wup = ctx.enter_context(tc.tile_pool(name='wup', bufs=3))
            u512 = ctx.enter_context(tc.tile_pool(name='u512', bufs=6,
                                                  space='PSUM'))

            # XF: [128(d within tile), ko, j, l]  fp16
            XF = usb.tile([P, KO, 4, L], F16, tag='XF')
            for i in range(2):
                for r in range(N_CORES):
                    (nc.sync if r % 2 == 0 else nc.scalar).dma_start(
                        XF[:, 4 * i:4 * i + 4, r // 2,
                           (r % 2) * LL:(r % 2) * LL + LL],
                        agf_out[i][r])
            if 'xf' in taps:
                d = nc.dram_tensor('tap_xf', [P, KO, 4, L], F16,
                                   kind="ExternalOutput")
                nc.sync.dma_start(d[:], XF[:])

            expu = usb.tile([P, NVCH, 16, VC], F16, tag='expu')
            dacc = usb.tile([P, NVCH, 16], F32, tag='dacc')
            for vc in range(NVCH):
                wut = wup.tile([P, KO, VC], F16, tag='wut')
                (nc.sync if vc % 2 == 0 else nc.scalar).dma_start(
                    wut[:], wuT[vc])
                for j in range(4):
                    for lt in range(4):
                        jlt = 4 * j + lt
                        ps = u512.tile([P, VC], F32, tag='u512')
                        for k in range(KO):
                            nc.tensor.matmul(
                                ps[:], XF[:, k, j, lt * P:(lt + 1) * P],
                                wut[:, k, :], start=(k == 0),
                                stop=(k == KO - 1))
                        nc.scalar.activation(
                            expu[:, vc, jlt, :], ps[:], AF.Exp,
                            accum_out=dacc[:, vc, jlt:jlt + 1])
            # reduce deno over vocab chunks, AllReduce over cores
            d4 = usb.tile([P, 4, 16], F32, tag='d4')
            nc.vector.tensor_tensor(d4[:], dacc[:, 0:4, :], dacc[:, 4:8, :],
                                    mybir.AluOpType.add)
            d2 = usb.tile([P, 2, 16], F32, tag='d2')
            nc.vector.tensor_tensor(d2[:], d4[:, 0:2, :], d4[:, 2:4, :],
                                    mybir.AluOpType.add)
            deno = usb.tile([P, 16], F32, tag='deno')
            nc.vector.tensor_tensor(deno[:], d2[:, 0, :], d2[:, 1, :],
                                    mybir.AluOpType.add)
            nc.gpsimd.dma_start(arin[:], deno[:])
            nc.gpsimd.collective_compute(
                "AllReduce", mybir.AluOpType.add,
                ins=[arin[:]], outs=[arout[:]], replica_groups=ALL_GROUP)
            denof = usb.tile([P, 16], F32, tag='denof')
            nc.sync.dma_start(denof[:], arout[:])
            binv = usb.tile([P, 16], F32, tag='binv')
            nc.vector.reciprocal_approx_fast(binv[:], denof[:])
            if 'deno' in taps:
                d = nc.dram_tensor('tap_deno', [P, 16], F32,
                                   kind="ExternalOutput")
                nc.sync.dma_start(d[:], denof[:])

            # normalize + store (split across engines/queues)
            dmae = [nc.sync, nc.gpsimd, nc.scalar]
            for vc in range(NVCH):
                for j in range(4):
                    for lt in range(4):
                        jlt = 4 * j + lt
                        res = ures_p.tile([P, VC], F16, tag='ures')
                        i = (vc * 16 + jlt) % 3
                        if i == 0:
                            nc.vector.tensor_tensor(
                                res[:], expu[:, vc, jlt, :],
                                binv[:, jlt:jlt + 1].to_broadcast((P, VC)),
                                mybir.AluOpType.mult)
                        elif i == 1:
                            nc.scalar.activation(
                                res[:], expu[:, vc, jlt, :], AF.Copy,
                                scale=binv[:, jlt:jlt + 1])
                        else:
                            nc.gpsimd.tensor_tensor(
                                res[:], expu[:, vc, jlt, :],
                                binv[:, jlt:jlt + 1].to_broadcast((P, VC)),
                                mybir.AluOpType.mult)
                        dmae[(vc * 16 + jlt) % 3].dma_start(
                            outp[vc, j, lt, :, :], res[:])

    nc.compile()
    return nc


# ----------------------------------------------------------------------------
# host-side prep
# ----------------------------------------------------------------------------

def _to_kimaj(a):
    """[K, M] -> [128, K//128, M] with K = ko*128 + ki."""
    K, M = a.shape
    return np.ascontiguousarray(
        a.reshape(K // P, P, M).transpose(1, 0, 2))


def prep_inputs(inputs):
    f = lambda k: np.asarray(inputs[k], dtype=np.float32)
    We, Wp, Wu = f('We'), f('Wp'), f('Wu')
    x = np.asarray(inputs['x']).astype(np.int64)
    z = np.asarray(inputs['z']).astype(np.int64)

    shared = {}
    for pfx, nl in (('enc', LENC), ('dec', LDEC)):
        Wq, Wk, Wv = f(pfx + '_Wq'), f(pfx + '_Wk'), f(pfx + '_Wv')
        Wo, W1, W2 = f(pfx + '_Wo'), f(pfx + '_W1'), f(pfx + '_W2')
        wq, wk, wv, wo, w1, w2 = [], [], [], [], [], []
        for l in range(nl):
            qa = Wq[l].transpose(2, 0, 1).reshape(DE, H * DA) * (DA ** -0.5)
            ka = Wk[l].transpose(2, 0, 1).reshape(DE, H * DA)
            va = Wv[l].transpose(2, 0, 1).reshape(DE, H * DA)
            wq.append(_to_kimaj(qa))
            wk.append(_to_kimaj(ka))
            wv.append(_to_kimaj(va))
            wo.append(_to_kimaj(Wo[l].T))
            w1k = _to_kimaj(W1[l].T)          # [128, 8, 4096]
            w1.append(np.ascontiguousarray(
                w1k.reshape(P, KO, 8, 512).transpose(2, 0, 1, 3)))
            w2k = _to_kimaj(W2[l].T)          # [128, 32, 1024]
            w2.append(np.ascontiguousarray(
                w2k.reshape(P, MO, 8, P).transpose(2, 0, 1, 3)))
        shared[f'{pfx}_wqT'] = np.stack(wq).astype(np.float16)
        shared[f'{pfx}_wkT'] = np.stack(wk).astype(np.float16)
        shared[f'{pfx}_wvT'] = np.stack(wv).astype(np.float16)
        shared[f'{pfx}_woT'] = np.stack(wo).astype(np.float16)
        shared[f'{pfx}_w1T'] = np.stack(w1).astype(np.float16)
        shared[f'{pfx}_w2T'] = np.stack(w2).astype(np.float16)

    pos = Wp[:L]  # [512, 1024]
    in_maps = []
    for c in range(N_CORES):
        b, h = c // 2, c % 2
        m = dict(shared)
        for nm, tok in (('z0', z[b]), ('x0', x[b])):
            E0 = (We[tok] + pos).T.astype(np.float32)      # [1024, 512]
            E0k = E0.reshape(KO, P, L)                     # [ko, ki, p]
            loc = E0k[:, :, h * LL:(h + 1) * LL].transpose(1, 0, 2)
            m[nm + '_loc32'] = np.ascontiguousarray(loc)
            m[nm + '_loc16'] = np.ascontiguousarray(loc).astype(np.float16)
        kglob = np.arange(L)[:, None]
        qglob = (h * LL + np.arange(LL))[None, :]
        msk = (kglob <= qglob).astype(np.float16)          # [512, 256]
        m['mask_self'] = np.ascontiguousarray(
            msk.reshape(KT, P, LL).transpose(1, 0, 2))
        wus = Wu[c * NVC:(c + 1) * NVC].T                  # [1024, 4000]
        wuk = _to_kimaj(wus)                               # [128, 8, 4000]
        m['wuT'] = np.ascontiguousarray(
            wuk.reshape(P, KO, NVCH, VC).transpose(2, 0, 1, 3)
        ).astype(np.float16)
        in_maps.append(m)
    return in_maps


def assemble(results):
    """results: per-core dicts with 'outp' [4, 4, 128, NVCH, VC] fp16."""
    out = np.empty((4, NV, L), dtype=np.float32)
    for c, r in enumerate(results):
        o = np.asarray(r['outp'], dtype=np.float32)  # [vc, j, lt, ll, vv]
        o = o.transpose(1, 0, 4, 2, 3).reshape(4, NVC, L)
        out[:, c * NVC:(c + 1) * NVC, :] = o
    return out


def run(inputs, trace=False, taps=(), trace_kwargs=None):
    key = ('prog', tuple(sorted(taps)))
    if key not in _CACHE:
        _CACHE[key] = build_program(taps=taps)
    nc = _CACHE[key]
    in_maps = prep_inputs(inputs)
    res = run_bass_kernel_spmd(nc, in_maps, list(range(N_CORES)),
                               trace=trace, **(trace_kwargs or {}))
    return res


def kernel(**inputs):
    res = run(inputs, trace=False)
    return assemble(res.results)


# revision 25
# speedup vs baseline: 1.2619x; 1.0875x over previous
"""Trainium2 Bass kernel for nn_EDTransformer (encoder-decoder transformer).

Sharding: 8 cores = 4 batch items x 2 sequence halves.
 - Each core owns (item b, half h): computes Q/scores/AV/Wo/MLP/LN for its
   256 local positions; K/V computed for the LOCAL half only and completed
   via a 2-core AllGather of K/V per attention block.
 - Decoder self+cross attention share one weight load per layer.
 - Unembedding sharded over vocab (4000 rows/core, 8 chunks of 500),
   computed TRANSPOSED (positions on partitions) so the softmax denominator
   comes from the Act engine accumulator and the normalize is a
   per-partition scale; denominator summed via one 8-core AllReduce.
 - Weights pre-tiled host-side so every DMA reads contiguous >=2KB runs
   per partition; weight loads spread across sync/scalar queues.
Dtypes: fp16 matmul operands, fp32 PSUM, fp32 residual + LN stats,
 fp16 output (cast to fp32 on host).
"""
import os
import sys

sys.path.insert(0, '/opt/trn_rl_repo')
import numpy as np

import concourse.bacc as bacc
import concourse.tile as tile
import concourse.mybir as mybir
from concourse.bass_utils import run_bass_kernel_spmd

DT = mybir.dt
F16 = DT.float16
F32 = DT.float32
AF = mybir.ActivationFunctionType

N_CORES = 8
P = 128
DE = 1024           # model dim (8 ptiles)
KO = DE // P        # 8
DMLP = 4096         # mlp dim
MO = DMLP // P      # 32
H = 16              # heads
DA = 64             # attn dim per head
L = 512             # sequence length
LL = 256            # local positions per core
KT = L // P         # 4 key tiles
NV = 32000
NVC = NV // N_CORES  # 4000 vocab rows per core
VC = 500            # vocab chunk (8 chunks of 500)
NVCH = NVC // VC    # 8
LENC = 2
LDEC = 2
EPS = 1e-5

PAIR_GROUPS = [[0, 1], [2, 3], [4, 5], [6, 7]]
ALL_GROUP = [list(range(N_CORES))]

_CACHE = {}


# ----------------------------------------------------------------------------
# device program
# ----------------------------------------------------------------------------

def _kv_proj_ag(nc, pools, W, kvin16, agin, agout, kfull, vt16):
    """Project K/V from local stream and pair-AllGather to full length.

    kvin16: [128, KO, LL] local stream.
    kfull : [128, KO, L]  (partitions = 2h x 64a rows)
    vt16  : [128, KT, H*DA] (partitions = key positions)
    """
    p256 = pools['p256']
    p512 = pools['p512']
    kloc = pools['att'].tile([P, KO, LL], F16, tag='kloc')
    for pr in range(KO):
        ps = p256.tile([P, LL], F32, tag='p256')
        for k in range(KO):
            nc.tensor.matmul(ps[:], W['wk'][:, k, pr * P:(pr + 1) * P],
                             kvin16[:, k, :], start=(k == 0), stop=(k == KO - 1))
        nc.vector.tensor_copy(kloc[:, pr, :], ps[:])
    vloc = pools['att'].tile([P, 2, H * DA], F16, tag='vloc')
    for lc in range(2):
        for nch in range(2):
            ps = p512.tile([P, 512], F32, tag='p512', bufs=2)
            for k in range(KO):
                nc.tensor.matmul(ps[:], kvin16[:, k, lc * P:(lc + 1) * P],
                                 W['wv'][:, k, nch * 512:(nch + 1) * 512],
                                 start=(k == 0), stop=(k == KO - 1))
            nc.vector.tensor_copy(vloc[:, lc, nch * 512:(nch + 1) * 512], ps[:])
    # stage k (2048) + v (2048) into one dram buffer, AllGather over the pair
    nc.gpsimd.dma_start(
        agin[:, 0:2048].rearrange('p (a b) -> p a b', a=KO), kloc[:])
    nc.gpsimd.dma_start(
        agin[:, 2048:4096].rearrange('p (a b) -> p a b', a=2), vloc[:])
    nc.gpsimd.collective_compute(
        "AllGather", mybir.AluOpType.bypass,
        ins=[agin[:]], outs=[agout[:]], replica_groups=PAIR_GROUPS)
    for r in range(2):
        nc.gpsimd.dma_start(
            kfull[:, :, r * LL:(r + 1) * LL],
            agout[r, :, 0:2048].rearrange('p (a b) -> p a b', a=KO))
        nc.gpsimd.dma_start(
            vt16[:, r * 2:(r + 1) * 2, :],
            agout[r, :, 2048:4096].rearrange('p (a b) -> p a b', a=2))


def _attn_core(nc, pools, Eres32, qin16, W, kfull, vt16, mask, name):
    """Q projection, scores/softmax/AV per head-pair, Wo accumulate."""
    p256 = pools['p256']
    p512 = pools['p512']
    ones = pools['ones']
    sb = pools['att']

    q16 = sb.tile([P, KO, LL], F16, tag='q16')
    for pr in range(KO):
        ps = p256.tile([P, LL], F32, tag='p256')
        for k in range(KO):
            nc.tensor.matmul(ps[:], W['wq'][:, k, pr * P:(pr + 1) * P],
                             qin16[:, k, :], start=(k == 0), stop=(k == KO - 1))
        nc.vector.tensor_copy(q16[:, pr, :], ps[:])

    y16 = sb.tile([P, KO, LL], F16, tag='y16')
    for pr in range(KO):
        hA, hB = 2 * pr, 2 * pr + 1
        # scores -> exp, 2 heads x 4 kt; psum pairs give [128, 512] exps
        exp16 = sb.tile([P, 2, KT, LL], F16, tag='exp16', bufs=2)
        for hh in range(2):
            h = hA + hh
            hp = (h % 2) * DA
            for kp in range(2):
                ps = p512.tile([P, 2, LL], F32, tag='psc', bufs=2)
                for ki in range(2):
                    kt = 2 * kp + ki
                    nc.tensor.matmul(
                        ps[:, ki, :],
                        kfull[hp:hp + DA, pr, kt * P:(kt + 1) * P],
                        q16[hp:hp + DA, pr, :], start=True, stop=True)
                nc.scalar.activation(exp16[:, hh, 2 * kp:2 * kp + 2, :],
                                     ps[:], AF.Exp)
        if mask is not None:
            nc.vector.tensor_tensor(
                exp16[:], exp16[:],
                mask[:, None, :, :].to_broadcast((P, 2, KT, LL)),
                mybir.AluOpType.mult)
        # denominators (replicated over partitions via ones matmul)
        pd = p512.tile([P, 2, LL], F32, tag='pd', bufs=2)
        for kt in range(KT):
            nc.tensor.matmul(pd[:], ones[:, :], exp16[:, :, kt, :],
                             start=(kt == 0), stop=(kt == KT - 1))
        ysc = pools['stat'].tile([P, 2, LL], F32, tag='ysc', bufs=2)
        nc.vector.reciprocal_approx_fast(ysc[:], pd[:])
        # AV (2-head column packing)
        ps = p256.tile([P, LL], F32, tag='p256')
        for kt in range(KT):
            nc.tensor.matmul(ps[:DA, :], vt16[:, kt, hA * DA:(hA + 1) * DA],
                             exp16[:, 0, kt, :], start=(kt == 0),
                             stop=(kt == KT - 1), tile_position=(0, 0))
            nc.tensor.matmul(ps[DA:, :], vt16[:, kt, hB * DA:(hB + 1) * DA],
                             exp16[:, 1, kt, :], start=(kt == 0),
                             stop=(kt == KT - 1), tile_position=(0, DA))
        nc.vector.tensor_tensor(y16[:DA, pr, :], ps[:DA, :], ysc[:DA, 0, :],
                                mybir.AluOpType.mult)
        nc.vector.tensor_tensor(y16[DA:, pr, :], ps[DA:, :], ysc[DA:, 1, :],
                                mybir.AluOpType.mult)

    # Wo -> accumulate into residual
    for dt in range(KO):
        ps = p256.tile([P, LL], F32, tag='p256')
        for k in range(KO):
            nc.tensor.matmul(ps[:], W['wo'][:, k, dt * P:(dt + 1) * P],
                             y16[:, k, :], start=(k == 0), stop=(k == KO - 1))
        nc.vector.tensor_tensor(Eres32[:, dt, :], Eres32[:, dt, :], ps[:],
                                mybir.AluOpType.add)
    tp = pools.get('tapfn')
    if tp:
        tp(f'{name}_q', q16)
        tp(f'{name}_y', y16)


def _load_attn_w(nc, pools, wq_d, wk_d, wv_d, wo_d):
    wq = pools['wqp'].tile([P, KO, DE], F16, tag='wq')
    nc.sync.dma_start(wq[:], wq_d[:])
    wk = pools['wkp'].tile([P, KO, DE], F16, tag='wk')
    nc.scalar.dma_start(wk[:], wk_d[:])
    wv = pools['wvp'].tile([P, KO, DE], F16, tag='wv')
    nc.sync.dma_start(wv[:], wv_d[:])
    wo = pools['wop'].tile([P, KO, DE], F16, tag='wo')
    nc.scalar.dma_start(wo[:], wo_d[:])
    return {'wq': wq, 'wk': wk, 'wv': wv, 'wo': wo}


def _mlp(nc, pools, Eres32, ein16, w1_d, w2_d, name):
    p256 = pools['p256']
    h16 = pools['mlp'].tile([P, MO, LL], F16, tag='h16')
    for c in range(8):
        w1t = pools['w1p'].tile([P, KO, 512], F16, tag='w1t')
        (nc.sync if c % 2 == 0 else nc.scalar).dma_start(w1t[:], w1_d[c])
        for m in range(4):
            mt = 4 * c + m
            ps = p256.tile([P, LL], F32, tag='p256')
            for k in range(KO):
                nc.tensor.matmul(ps[:], w1t[:, k, m * P:(m + 1) * P],
                                 ein16[:, k, :], start=(k == 0),
                                 stop=(k == KO - 1))
            nc.scalar.activation(h16[:, mt, :], ps[:], AF.Relu)
    for dt in range(KO):
        w2t = pools['w2p'].tile([P, MO, P], F16, tag='w2t')
        (nc.sync if dt % 2 == 0 else nc.scalar).dma_start(w2t[:], w2_d[dt])
        ps = p256.tile([P, LL], F32, tag='p256')
        for k in range(MO):
            nc.tensor.matmul(ps[:], w2t[:, k, :], h16[:, k, :],
                             start=(k == 0), stop=(k == MO - 1))
        nc.vector.tensor_tensor(Eres32[:, dt, :], Eres32[:, dt, :], ps[:],
                                mybir.AluOpType.add)


def _ln(nc, pools, Eres32, e16out, name):
    """In-place layernorm over features; writes fp16 copy to e16out."""
    p256 = pools['p256']
    ones = pools['ones']
    stat = pools['stat']

    e16pre = pools['lnp'].tile([P, KO, LL], F16, tag='e16pre')
    nc.vector.tensor_copy(e16pre[:], Eres32[:])
    sq16 = pools['lnp'].tile([P, KO, LL], F16, tag='sq16')
    nc.scalar.square(sq16[:], e16pre[:])
    pss = p256.tile([P, LL], F32, tag='p256')
    psq = p256.tile([P, LL], F32, tag='p256')
    for k in range(KO):
        nc.tensor.matmul(pss[:], ones[:, :], e16pre[:, k, :],
                         start=(k == 0), stop=(k == KO - 1))
    for k in range(KO):
        nc.tensor.matmul(psq[:], ones[:, :], sq16[:, k, :],
                         start=(k == 0), stop=(k == KO - 1))
    mean = stat.tile([P, LL], F32, tag='mean')
    nc.vector.tensor_scalar_mul(mean[:], pss[:], 1.0 / DE)
    varn = stat.tile([P, LL], F32, tag='varn')
    nc.vector.tensor_tensor(varn[:], pss[:], mean[:], mybir.AluOpType.mult)
    nc.vector.tensor_tensor(varn[:], psq[:], varn[:], mybir.AluOpType.subtract)
    std = stat.tile([P, LL], F32, tag='std')
    nc.scalar.activation(std[:], varn[:], AF.Sqrt,
                         bias=pools['eps128'], scale=1.0 / (DE - 1))
    inv = stat.tile([P, LL], F32, tag='inv')
    nc.vector.reciprocal_approx_fast(inv[:], std[:])
    ms = stat.tile([P, LL], F32, tag='ms')
    nc.vector.tensor_tensor(ms[:], mean[:], inv[:], mybir.AluOpType.mult)
    nc.vector.tensor_tensor(
        Eres32[:], Eres32[:],
        inv[:, None, :].to_broadcast((P, KO, LL)), mybir.AluOpType.mult)
    nc.vector.tensor_tensor(
        Eres32[:], Eres32[:],
        ms[:, None, :].to_broadcast((P, KO, LL)), mybir.AluOpType.subtract)
    nc.vector.tensor_copy(e16out[:], Eres32[:])
    tp = pools.get('tapfn')
    if tp:
        tp(f'{name}_out', Eres32)


def build_program(taps=()):
    taps = set(taps)
    nc = bacc.Bacc("TRN2", target_bir_lowering=False, debug=False,
                   num_devices=N_CORES)

    # ---- dram inputs ----
    din = {}

    def dram_in(nm, shape, dt=F16):
        din[nm] = nc.dram_tensor(nm, list(shape), dt, kind="ExternalInput")
        return din[nm]

    z0l32 = dram_in('z0_loc32', [P, KO, LL], F32)
    x0l32 = dram_in('x0_loc32', [P, KO, LL], F32)
    z0l16 = dram_in('z0_loc16', [P, KO, LL])
    x0l16 = dram_in('x0_loc16', [P, KO, LL])
    mask_self = dram_in('mask_self', [P, KT, LL])
    for pfx, nl in (('enc', LENC), ('dec', LDEC)):
        for w in ('wqT', 'wkT', 'wvT', 'woT'):
            dram_in(f'{pfx}_{w}', [nl, P, KO, DE])
        dram_in(f'{pfx}_w1T', [nl, 8, P, KO, 512])
        dram_in(f'{pfx}_w2T', [nl, 8, P, MO, P])
    wuT = dram_in('wuT', [NVCH, P, KO, VC])

    # output: [vc, ll, j, lt, vv] fp16 (one 2MB store per vc, 16KB rows)
    outp = nc.dram_tensor('outp', [NVCH, P, 16, VC], F16,
                          kind="ExternalOutput")

    # internal dram for collectives (reused across attns; gpsimd-serialized)
    agin = nc.dram_tensor('agin', [P, 4096], F16)
    agout = nc.dram_tensor('agout', [2, P, 4096], F16)
    agf_in = [nc.dram_tensor(f'agf_in{i}', [P, 4, LL], F16) for i in range(2)]
    agf_out = [nc.dram_tensor(f'agf_out{i}', [N_CORES, P, 4, LL], F16,
                              addr_space='Shared') for i in range(2)]
    arin = nc.dram_tensor('arin', [P, 16], F32)
    arout = nc.dram_tensor('arout', [P, 16], F32, addr_space="Shared")
    # tiny warm-up buffers: trigger CC channel init at t=0
    wu_in = nc.dram_tensor('wu_in', [P, 1], F16)
    wu_p_out = nc.dram_tensor('wu_p_out', [2, P, 1], F16)
    wu_a_out = nc.dram_tensor('wu_a_out', [N_CORES, P, 1], F16,
                              addr_space='Shared')
    wu_r_in = nc.dram_tensor('wu_r_in', [P, 1], F32)
    wu_r_out = nc.dram_tensor('wu_r_out', [P, 1], F32, addr_space='Shared')

    import contextlib
    with tile.TileContext(nc) as tc, contextlib.ExitStack() as octx:
        const = octx.enter_context(tc.tile_pool(name='const', bufs=1))
        ones = const.tile([P, P], F16)
        nc.vector.memset(ones[:], 1.0)
        eps128 = const.tile([P, 1], F32)
        nc.vector.memset(eps128[:], EPS)
        msk = const.tile([P, KT, LL], F16)
        nc.sync.dma_start(msk[:], mask_self[:])

        # warm up the pair CC channels immediately; all-group channels are
        # warmed later (mid-decoder) to keep them off the early CC queue
        wtile = const.tile([P, 1], F16)
        nc.vector.memset(wtile[:], 0.0)
        nc.gpsimd.dma_start(wu_in[:], wtile[:])
        wtile32 = const.tile([P, 1], F32)
        nc.vector.memset(wtile32[:], 0.0)
        nc.gpsimd.dma_start(wu_r_in[:], wtile32[:])
        nc.gpsimd.collective_compute(
            "AllGather", mybir.AluOpType.bypass,
            ins=[wu_in[:]], outs=[wu_p_out[:]], replica_groups=PAIR_GROUPS)

        def warm_all_group():
            nc.gpsimd.collective_compute(
                "AllGather", mybir.AluOpType.bypass,
                ins=[wu_in[:]], outs=[wu_a_out[:]], replica_groups=ALL_GROUP)
            nc.gpsimd.collective_compute(
                "AllReduce", mybir.AluOpType.add,
                ins=[wu_r_in[:]], outs=[wu_r_out[:]], replica_groups=ALL_GROUP)

        # ================= layer phase =================
        with contextlib.ExitStack() as ctx:
            stream = ctx.enter_context(tc.tile_pool(name='stream', bufs=1))
            att = ctx.enter_context(tc.tile_pool(name='att', bufs=1))
            mlpp = ctx.enter_context(tc.tile_pool(name='mlpp', bufs=1))
            lnp = ctx.enter_context(tc.tile_pool(name='lnp', bufs=1))
            stat = ctx.enter_context(tc.tile_pool(name='stat', bufs=1))
            wqp = ctx.enter_context(tc.tile_pool(name='wqp', bufs=1))
            wkp = ctx.enter_context(tc.tile_pool(name='wkp', bufs=1))
            wvp = ctx.enter_context(tc.tile_pool(name='wvp', bufs=1))
            wop = ctx.enter_context(tc.tile_pool(name='wop', bufs=1))
            w1p = ctx.enter_context(tc.tile_pool(name='w1p', bufs=2))
            w2p = ctx.enter_context(tc.tile_pool(name='w2p', bufs=2))
            p256 = ctx.enter_context(tc.tile_pool(name='p256', bufs=2,
                                                  space='PSUM'))
            p512 = ctx.enter_context(tc.tile_pool(name='p512', bufs=4,
                                                  space='PSUM'))

            pools = dict(att=att, mlp=mlpp, lnp=lnp, p256=p256, p512=p512,
                         stat=stat, ones=ones, eps128=eps128[:],
                         wqp=wqp, wkp=wkp, wvp=wvp, wop=wop, w1p=w1p, w2p=w2p)

            def tapfn(nm, t):
                if nm not in taps:
                    return
                d = nc.dram_tensor('tap_' + nm, list(t.shape),
                                   t.dtype, kind="ExternalOutput")
                nc.sync.dma_start(d[:], t[:])
            pools['tapfn'] = tapfn

            # ======== encoder ========
            Eres = stream.tile([P, KO, LL], F32, tag='res')
            nc.sync.dma_start(Eres[:], z0l32[:])
            eloc = stream.tile([P, KO, LL], F16, tag='loc_a')
            nc.sync.dma_start(eloc[:], z0l16[:])

            for l in range(LENC):
                W = _load_attn_w(nc, pools, din['enc_wqT'][l],
                                 din['enc_wkT'][l], din['enc_wvT'][l],
                                 din['enc_woT'][l])
                kfull = att.tile([P, KO, L], F16, tag='kfull')
                vt16 = att.tile([P, KT, H * DA], F16, tag='vt16')
                _kv_proj_ag(nc, pools, W, eloc, agin, agout, kfull, vt16)
                _attn_core(nc, pools, Eres, eloc, W, kfull, vt16, None,
                           f'e{l}a')
                eloc = stream.tile([P, KO, LL], F16, tag='loc_b')
                _ln(nc, pools, Eres, eloc, f'e{l}ln1')
                _mlp(nc, pools, Eres, eloc, din['enc_w1T'][l],
                     din['enc_w2T'][l], f'e{l}m')
                eloc = stream.tile([P, KO, LL], F16, tag='loc_a')
                _ln(nc, pools, Eres, eloc, f'e{l}ln2')

            Zloc = stream.tile([P, KO, LL], F16, tag='zloc')
            nc.vector.tensor_copy(Zloc[:], eloc[:])

            # ======== decoder ========
            Eres = stream.tile([P, KO, LL], F32, tag='res')
            nc.sync.dma_start(Eres[:], x0l32[:])
            eloc = stream.tile([P, KO, LL], F16, tag='loc_a')
            nc.sync.dma_start(eloc[:], x0l16[:])

            for l in range(LDEC):
                W = _load_attn_w(nc, pools, din['dec_wqT'][l],
                                 din['dec_wkT'][l], din['dec_wvT'][l],
                                 din['dec_woT'][l])
                # self K/V + AG
                kfull_s = att.tile([P, KO, L], F16, tag='kfull')
                vt16_s = att.tile([P, KT, H * DA], F16, tag='vt16')
                _kv_proj_ag(nc, pools, W, eloc, agin, agout, kfull_s, vt16_s)
                # cross K/V + AG (overlaps self AG; weights shared)
                kfull_c = att.tile([P, KO, L], F16, tag='kfull_c')
                vt16_c = att.tile([P, KT, H * DA], F16, tag='vt16_c')
                _kv_proj_ag(nc, pools, W, Zloc, agin, agout, kfull_c, vt16_c)
                if l == 0:
                    warm_all_group()
                # self attention (causal)
                _attn_core(nc, pools, Eres, eloc, W, kfull_s, vt16_s, msk,
                           f'd{l}s')
                eloc = stream.tile([P, KO, LL], F16, tag='loc_b')
                _ln(nc, pools, Eres, eloc, f'd{l}ln1')
                # cross attention
                _attn_core(nc, pools, Eres, eloc, W, kfull_c, vt16_c, None,
                           f'd{l}c')
                eloc = stream.tile([P, KO, LL], F16, tag='loc_b')
                _ln(nc, pools, Eres, eloc, f'd{l}ln2')
                _mlp(nc, pools, Eres, eloc, din['dec_w1T'][l],
                     din['dec_w2T'][l], f'd{l}m')
                eloc = stream.tile([P, KO, LL], F16, tag='loc_a')
                _ln(nc, pools, Eres, eloc, f'd{l}ln3')

            # final activations for the 8-core gather, in 2 chunks
            for i in range(2):
                nc.gpsimd.dma_start(agf_in[i][:], eloc[:, 4 * i:4 * i + 4, :])
                nc.gpsimd.collective_compute(
                    "AllGather", mybir.AluOpType.bypass,
                    ins=[agf_in[i][:]], outs=[agf_out[i][:]],
                    replica_groups=ALL_GROUP)

        # ================= unembed phase (transposed) =================
        with contextlib.ExitStack() as ctx:
            usb = ctx.enter_context(tc.tile_pool(name='usb', bufs=1))
            wup = ctx.enter_context(tc.tile_pool(name='wup', bufs=3))
            u512 = ctx.enter_context(tc.tile_pool(name='u512', bufs=6,
                                                  space='PSUM'))

            # XF: [128(d within tile), ko, j, l]  fp16
            XF = usb.tile([P, KO, 4, L], F16, tag='XF')
            for i in range(2):
                for r in range(N_CORES):
                    (nc.sync if r % 2 == 0 else nc.scalar).dma_start(
                        XF[:, 4 * i:4 * i + 4, r // 2,
                           (r % 2) * LL:(r % 2) * LL + LL],
                        agf_out[i][r])
            if 'xf' in taps:
                d = nc.dram_tensor('tap_xf', [P, KO, 4, L], F16,
                                   kind="ExternalOutput")
                nc.sync.dma_start(d[:], XF[:])

            expu = usb.tile([P, NVCH, 16, VC], F16, tag='expu')
            dacc = usb.tile([P, NVCH, 16], F32, tag='dacc')
            for vc in range(NVCH):
                wut = wup.tile([P, KO, VC], F16, tag='wut')
                (nc.sync if vc % 2 == 0 else nc.scalar).dma_start(
                    wut[:], wuT[vc])
                for j in range(4):
                    for lt in range(4):
                        jlt = 4 * j + lt
                        ps = u512.tile([P, VC], F32, tag='u512')
                        for k in range(KO):
                            nc.tensor.matmul(
                                ps[:], XF[:, k, j, lt * P:(lt + 1) * P],
                                wut[:, k, :], start=(k == 0),
                                stop=(k == KO - 1))
                        nc.scalar.activation(
                            expu[:, vc, jlt, :], ps[:], AF.Exp,
                            accum_out=dacc[:, vc, jlt:jlt + 1])
            # reduce deno over vocab chunks, AllReduce over cores
            d4 = usb.tile([P, 4, 16], F32, tag='d4')
            nc.vector.tensor_tensor(d4[:], dacc[:, 0:4, :], dacc[:, 4:8, :],
                                    mybir.AluOpType.add)
            d2 = usb.tile([P, 2, 16], F32, tag='d2')
            nc.vector.tensor_tensor(d2[:], d4[:, 0:2, :], d4[:, 2:4, :],
                                    mybir.AluOpType.add)
            deno = usb.tile([P, 16], F32, tag='deno')
            nc.vector.tensor_tensor(deno[:], d2[:, 0, :], d2[:, 1, :],
                                    mybir.AluOpType.add)
            nc.gpsimd.dma_start(arin[:], deno[:])
            nc.gpsimd.collective_compute(
                "AllReduce", mybir.AluOpType.add,
                ins=[arin[:]], outs=[arout[:]], replica_groups=ALL_GROUP)
            denof = usb.tile([P, 16], F32, tag='denof')
            nc.sync.dma_start(denof[:], arout[:])
            binv = usb.tile([P, 16], F32, tag='binv')
            nc.vector.reciprocal_approx_fast(binv[:], denof[:])
            if 'deno' in taps:
                d = nc.dram_tensor('tap_deno', [P, 16], F32,
                                   kind="ExternalOutput")
                nc.sync.dma_start(d[:], denof[:])

            # normalize in place (3 engines), then one big store per vc
            dmae = [nc.sync, nc.gpsimd, nc.scalar]
            for vc in range(NVCH):
                for jlt in range(16):
                    sl = expu[:, vc, jlt, :]
                    i = (vc * 16 + jlt) % 3
                    if i == 0:
                        nc.vector.tensor_tensor(
                            sl, sl, binv[:, jlt:jlt + 1].to_broadcast((P, VC)),
                            mybir.AluOpType.mult)
                    elif i == 1:
                        nc.scalar.activation(sl, sl, AF.Copy,
                                             scale=binv[:, jlt:jlt + 1])
                    else:
                        nc.gpsimd.tensor_tensor(
                            sl, sl, binv[:, jlt:jlt + 1].to_broadcast((P, VC)),
                            mybir.AluOpType.mult)
                dmae[vc % 3].dma_start(outp[vc], expu[:, vc, :, :])

    nc.compile()
    return nc


# ----------------------------------------------------------------------------
# host-side prep
# ----------------------------------------------------------------------------

def _to_kimaj(a):
    """[K, M] -> [128, K//128, M] with K = ko*128 + ki."""
    K, M = a.shape
    return np.ascontiguousarray(
        a.reshape(K // P, P, M).transpose(1, 0, 2))


def prep_inputs(inputs):
    f = lambda k: np.asarray(inputs[k], dtype=np.float32)
    We, Wp, Wu = f('We'), f('Wp'), f('Wu')
    x = np.asarray(inputs['x']).astype(np.int64)
    z = np.asarray(inputs['z']).astype(np.int64)

    shared = {}
    for pfx, nl in (('enc', LENC), ('dec', LDEC)):
        Wq, Wk, Wv = f(pfx + '_Wq'), f(pfx + '_Wk'), f(pfx + '_Wv')
        Wo, W1, W2 = f(pfx + '_Wo'), f(pfx + '_W1'), f(pfx + '_W2')
        wq, wk, wv, wo, w1, w2 = [], [], [], [], [], []
        for l in range(nl):
            qa = Wq[l].transpose(2, 0, 1).reshape(DE, H * DA) * (DA ** -0.5)
            ka = Wk[l].transpose(2, 0, 1).reshape(DE, H * DA)
            va = Wv[l].transpose(2, 0, 1).reshape(DE, H * DA)
            wq.append(_to_kimaj(qa))
            wk.append(_to_kimaj(ka))
            wv.append(_to_kimaj(va))
            wo.append(_to_kimaj(Wo[l].T))
            w1k = _to_kimaj(W1[l].T)          # [128, 8, 4096]
            w1.append(np.ascontiguousarray(
                w1k.reshape(P, KO, 8, 512).transpose(2, 0, 1, 3)))
            w2k = _to_kimaj(W2[l].T)          # [128, 32, 1024]
            w2.append(np.ascontiguousarray(
                w2k.reshape(P, MO, 8, P).transpose(2, 0, 1, 3)))
        shared[f'{pfx}_wqT'] = np.stack(wq).astype(np.float16)
        shared[f'{pfx}_wkT'] = np.stack(wk).astype(np.float16)
        shared[f'{pfx}_wvT'] = np.stack(wv).astype(np.float16)
        shared[f'{pfx}_woT'] = np.stack(wo).astype(np.float16)
        shared[f'{pfx}_w1T'] = np.stack(w1).astype(np.float16)
        shared[f'{pfx}_w2T'] = np.stack(w2).astype(np.float16)

    pos = Wp[:L]  # [512, 1024]
    in_maps = []
    for c in range(N_CORES):
        b, h = c // 2, c % 2
        m = dict(shared)
        for nm, tok in (('z0', z[b]), ('x0', x[b])):
            E0 = (We[tok] + pos).T.astype(np.float32)      # [1024, 512]
            E0k = E0.reshape(KO, P, L)                     # [ko, ki, p]
            loc = E0k[:, :, h * LL:(h + 1) * LL].transpose(1, 0, 2)
            m[nm + '_loc32'] = np.ascontiguousarray(loc)
            m[nm + '_loc16'] = np.ascontiguousarray(loc).astype(np.float16)
        kglob = np.arange(L)[:, None]
        qglob = (h * LL + np.arange(LL))[None, :]
        msk = (kglob <= qglob).astype(np.float16)          # [512, 256]
        m['mask_self'] = np.ascontiguousarray(
            msk.reshape(KT, P, LL).transpose(1, 0, 2))
        wus = Wu[c * NVC:(c + 1) * NVC].T                  # [1024, 4000]
        wuk = _to_kimaj(wus)                               # [128, 8, 4000]
        m['wuT'] = np.ascontiguousarray(
            wuk.reshape(P, KO, NVCH, VC).transpose(2, 0, 1, 3)
        ).astype(np.float16)
        in_maps.append(m)
    return in_maps


def assemble(results):
    """results: per-core dicts with 'outp' [4, 4, 128, NVCH, VC] fp16."""
    out = np.empty((4, NV, L), dtype=np.float32)
    for c, r in enumerate(results):
        o = np.asarray(r['outp'], dtype=np.float32)  # [vc, ll, jlt, vv]
        o = o.reshape(NVCH, P, 4, 4, VC).transpose(2, 0, 4, 3, 1)
        out[:, c * NVC:(c + 1) * NVC, :] = o.reshape(4, NVC, L)
    return out


def run(inputs, trace=False, taps=(), trace_kwargs=None):
    key = ('prog', tuple(sorted(taps)))
    if key not in _CACHE:
        _CACHE[key] = build_program(taps=taps)
    nc = _CACHE[key]
    in_maps = prep_inputs(inputs)
    res = run_bass_kernel_spmd(nc, in_maps, list(range(N_CORES)),
                               trace=trace, **(trace_kwargs or {}))
    return res


def kernel(**inputs):
    res = run(inputs, trace=False)
    return assemble(res.results)


# revision 42
# speedup vs baseline: 1.4089x; 1.1164x over previous
"""Trainium2 Bass kernel for nn_EDTransformer (encoder-decoder transformer).

Sharding: 8 cores = 4 batch items x 2 sequence halves.
 - Each core owns (item b, half h): computes Q/scores/AV/Wo/MLP/LN for its
   256 local positions; K/V computed for the LOCAL half only and completed
   via a 2-core AllGather of K/V per attention block.
 - Decoder self+cross attention share one weight load per layer.
 - Unembedding sharded over vocab (4000 rows/core, 8 chunks of 500),
   computed TRANSPOSED (positions on partitions) so the softmax denominator
   comes from the Act engine accumulator and the normalize is a
   per-partition scale; denominator summed via one 8-core AllReduce.
 - Weights pre-tiled host-side so every DMA reads contiguous >=2KB runs
   per partition; weight loads spread across sync/scalar queues.
Dtypes: fp16 matmul operands, fp32 PSUM, fp32 residual + LN stats,
 fp16 output (cast to fp32 on host).
"""
import os
import sys

sys.path.insert(0, '/opt/trn_rl_repo')
import numpy as np

import concourse.bacc as bacc
import concourse.tile as tile
import concourse.mybir as mybir
from concourse.bass_utils import run_bass_kernel_spmd

DT = mybir.dt
F16 = DT.float16
F32 = DT.float32
AF = mybir.ActivationFunctionType

N_CORES = 8
P = 128
DE = 1024           # model dim (8 ptiles)
KO = DE // P        # 8
DMLP = 4096         # mlp dim
MO = DMLP // P      # 32
H = 16              # heads
DA = 64             # attn dim per head
L = 512             # sequence length
LL = 256            # local positions per core
KT = L // P         # 4 key tiles
NV = 32000
NVC = NV // N_CORES  # 4000 vocab rows per core
VC = 500            # vocab chunk (8 chunks of 500)
NVCH = NVC // VC    # 8
LENC = 2
LDEC = 2
EPS = 1e-5

PAIR_GROUPS = [[0, 1], [2, 3], [4, 5], [6, 7]]
ALL_GROUP = [list(range(N_CORES))]

_CACHE = {}


# ----------------------------------------------------------------------------
# device program
# ----------------------------------------------------------------------------

def _kv_proj_ag(nc, pools, W, kvin16, agin, agout, kfull, vt16):
    """Project K/V from local stream and pair-AllGather to full length.

    kvin16: [128, KO, LL] local stream.
    kfull : [128, KO, L]  (partitions = 2h x 64a rows)
    vt16  : [128, KT, H*DA] (partitions = key positions)
    """
    p256 = pools['p256']
    p512 = pools['p512']
    kloc = pools['att'].tile([P, KO, LL], F16, tag='kloc')
    for pr in range(KO):
        ps = p256.tile([P, LL], F32, tag='p256')
        for k in range(KO):
            nc.tensor.matmul(ps[:], W['wk'][:, k, pr * P:(pr + 1) * P],
                             kvin16[:, k, :], start=(k == 0), stop=(k == KO - 1))
        nc.vector.tensor_copy(kloc[:, pr, :], ps[:])
    vloc = pools['att'].tile([P, 2, H * DA], F16, tag='vloc')
    for lc in range(2):
        for nch in range(2):
            ps = p512.tile([P, 512], F32, tag='p512', bufs=2)
            for k in range(KO):
                nc.tensor.matmul(ps[:], kvin16[:, k, lc * P:(lc + 1) * P],
                                 W['wv'][:, k, nch * 512:(nch + 1) * 512],
                                 start=(k == 0), stop=(k == KO - 1))
            nc.vector.tensor_copy(vloc[:, lc, nch * 512:(nch + 1) * 512], ps[:])
    # stage k (2048) + v (2048) into one dram buffer, AllGather over the pair
    nc.gpsimd.dma_start(
        agin[:, 0:2048].rearrange('p (a b) -> p a b', a=KO), kloc[:])
    nc.gpsimd.dma_start(
        agin[:, 2048:4096].rearrange('p (a b) -> p a b', a=2), vloc[:])
    nc.gpsimd.collective_compute(
        "AllGather", mybir.AluOpType.bypass,
        ins=[agin[:]], outs=[agout[:]], replica_groups=PAIR_GROUPS)
    for r in range(2):
        nc.gpsimd.dma_start(
            kfull[:, :, r * LL:(r + 1) * LL],
            agout[r, :, 0:2048].rearrange('p (a b) -> p a b', a=KO))
        nc.gpsimd.dma_start(
            vt16[:, r * 2:(r + 1) * 2, :],
            agout[r, :, 2048:4096].rearrange('p (a b) -> p a b', a=2))


def _attn_core(nc, pools, Eres32, qin16, W, kfull, vt16, mask, name):
    """Q projection, scores/softmax/AV per head-pair, Wo accumulate."""
    p256 = pools['p256']
    p512 = pools['p512']
    ones = pools['ones']
    sb = pools['att']

    q16 = sb.tile([P, KO, LL], F16, tag='q16')
    for pr in range(KO):
        ps = p256.tile([P, LL], F32, tag='p256')
        for k in range(KO):
            nc.tensor.matmul(ps[:], W['wq'][:, k, pr * P:(pr + 1) * P],
                             qin16[:, k, :], start=(k == 0), stop=(k == KO - 1))
        nc.vector.tensor_copy(q16[:, pr, :], ps[:])

    y16 = sb.tile([P, KO, LL], F16, tag='y16')
    for pr in range(KO):
        hA, hB = 2 * pr, 2 * pr + 1
        # scores -> exp, 2 heads x 4 kt; psum pairs give [128, 512] exps
        exp16 = sb.tile([P, 2, KT, LL], F16, tag='exp16', bufs=2)
        for hh in range(2):
            h = hA + hh
            hp = (h % 2) * DA
            for kp in range(2):
                ps = p512.tile([P, 2, LL], F32, tag='psc', bufs=2)
                for ki in range(2):
                    kt = 2 * kp + ki
                    nc.tensor.matmul(
                        ps[:, ki, :],
                        kfull[hp:hp + DA, pr, kt * P:(kt + 1) * P],
                        q16[hp:hp + DA, pr, :], start=True, stop=True)
                nc.scalar.activation(exp16[:, hh, 2 * kp:2 * kp + 2, :],
                                     ps[:], AF.Exp)
        if mask is not None:
            nc.vector.tensor_tensor(
                exp16[:], exp16[:],
                mask[:, None, :, :].to_broadcast((P, 2, KT, LL)),
                mybir.AluOpType.mult)
        # denominators (replicated over partitions via ones matmul)
        pd = p512.tile([P, 2, LL], F32, tag='pd', bufs=2)
        for kt in range(KT):
            nc.tensor.matmul(pd[:], ones[:, :], exp16[:, :, kt, :],
                             start=(kt == 0), stop=(kt == KT - 1))
        ysc = pools['stat'].tile([P, 2, LL], F32, tag='ysc', bufs=2)
        nc.vector.reciprocal_approx_fast(ysc[:], pd[:])
        # AV (2-head column packing)
        ps = p256.tile([P, LL], F32, tag='p256')
        for kt in range(KT):
            nc.tensor.matmul(ps[:DA, :], vt16[:, kt, hA * DA:(hA + 1) * DA],
                             exp16[:, 0, kt, :], start=(kt == 0),
                             stop=(kt == KT - 1), tile_position=(0, 0))
            nc.tensor.matmul(ps[DA:, :], vt16[:, kt, hB * DA:(hB + 1) * DA],
                             exp16[:, 1, kt, :], start=(kt == 0),
                             stop=(kt == KT - 1), tile_position=(0, DA))
        nc.vector.tensor_tensor(y16[:DA, pr, :], ps[:DA, :], ysc[:DA, 0, :],
                                mybir.AluOpType.mult)
        nc.vector.tensor_tensor(y16[DA:, pr, :], ps[DA:, :], ysc[DA:, 1, :],
                                mybir.AluOpType.mult)

    # Wo -> residual: Eres = psum + stream base (the attn input)
    for dt in range(KO):
        ps = p256.tile([P, LL], F32, tag='p256')
        for k in range(KO):
            nc.tensor.matmul(ps[:], W['wo'][:, k, dt * P:(dt + 1) * P],
                             y16[:, k, :], start=(k == 0), stop=(k == KO - 1))
        nc.vector.tensor_tensor(Eres32[:, dt, :], ps[:], qin16[:, dt, :],
                                mybir.AluOpType.add)
    tp = pools.get('tapfn')
    if tp:
        tp(f'{name}_q', q16)
        tp(f'{name}_y', y16)


def _load_attn_w(nc, pools, wq_d, wk_d, wv_d, wo_d):
    wq = pools['wqp'].tile([P, KO, DE], F16, tag='wq')
    nc.sync.dma_start(wq[:], wq_d[:])
    wk = pools['wkp'].tile([P, KO, DE], F16, tag='wk')
    nc.scalar.dma_start(wk[:], wk_d[:])
    wv = pools['wvp'].tile([P, KO, DE], F16, tag='wv')
    nc.sync.dma_start(wv[:], wv_d[:])
    wo = pools['wop'].tile([P, KO, DE], F16, tag='wo')
    nc.scalar.dma_start(wo[:], wo_d[:])
    return {'wq': wq, 'wk': wk, 'wv': wv, 'wo': wo}


def _mlp(nc, pools, Eres32, ein16, w1_d, w2_d, name):
    p256 = pools['p256']
    h16 = pools['mlp'].tile([P, MO, LL], F16, tag='h16')
    for c in range(16):
        w1t = pools['w1p'].tile([P, KO, LL], F16, tag='w1t')
        (nc.sync if c % 2 == 0 else nc.scalar).dma_start(w1t[:], w1_d[c])
        for m in range(2):
            mt = 2 * c + m
            ps = p256.tile([P, LL], F32, tag='p256')
            for k in range(KO):
                nc.tensor.matmul(ps[:], w1t[:, k, m * P:(m + 1) * P],
                                 ein16[:, k, :], start=(k == 0),
                                 stop=(k == KO - 1))
            nc.scalar.activation(h16[:, mt, :], ps[:], AF.Relu)
    for dt in range(KO):
        ps = p256.tile([P, LL], F32, tag='p256')
        for half in range(2):
            w2t = pools['w2p'].tile([P, 16, P], F16, tag='w2t')
            (nc.sync if half == 0 else nc.scalar).dma_start(
                w2t[:], w2_d[2 * dt + half])
            for k in range(16):
                nc.tensor.matmul(ps[:], w2t[:, k, :],
                                 h16[:, half * 16 + k, :],
                                 start=(half == 0 and k == 0),
                                 stop=(half == 1 and k == 15))
        nc.vector.tensor_tensor(Eres32[:, dt, :], ps[:], ein16[:, dt, :],
                                mybir.AluOpType.add)


def _ln(nc, pools, Eres32, e16out, name):
    """In-place layernorm over features; writes fp16 copy to e16out."""
    p256 = pools['p256']
    ones = pools['ones']
    stat = pools['stat']

    e16pre = pools['lnp'].tile([P, KO, LL], F16, tag='e16pre')
    nc.vector.tensor_copy(e16pre[:], Eres32[:])
    sq16 = pools['lnp'].tile([P, KO, LL], F16, tag='sq16')
    nc.scalar.square(sq16[:], e16pre[:])
    pss = p256.tile([P, LL], F32, tag='p256')
    psq = p256.tile([P, LL], F32, tag='p256')
    for k in range(KO):
        nc.tensor.matmul(pss[:], ones[:, :], e16pre[:, k, :],
                         start=(k == 0), stop=(k == KO - 1))
    for k in range(KO):
        nc.tensor.matmul(psq[:], ones[:, :], sq16[:, k, :],
                         start=(k == 0), stop=(k == KO - 1))
    mean = stat.tile([P, LL], F32, tag='mean')
    nc.vector.tensor_scalar_mul(mean[:], pss[:], 1.0 / DE)
    varn = stat.tile([P, LL], F32, tag='varn')
    nc.vector.tensor_tensor(varn[:], pss[:], mean[:], mybir.AluOpType.mult)
    nc.vector.tensor_tensor(varn[:], psq[:], varn[:], mybir.AluOpType.subtract)
    std = stat.tile([P, LL], F32, tag='std')
    nc.scalar.activation(std[:], varn[:], AF.Sqrt,
                         bias=pools['eps128'], scale=1.0 / (DE - 1))
    inv = stat.tile([P, LL], F32, tag='inv')
    nc.vector.reciprocal_approx_fast(inv[:], std[:])
    ms = stat.tile([P, LL], F32, tag='ms')
    nc.vector.tensor_tensor(ms[:], mean[:], inv[:], mybir.AluOpType.mult)
    t16 = pools['lnp'].tile([P, KO, LL], F16, tag='sq16')
    nc.vector.tensor_tensor(
        t16[:], Eres32[:],
        inv[:, None, :].to_broadcast((P, KO, LL)), mybir.AluOpType.mult)
    nc.vector.tensor_tensor(
        e16out[:], t16[:],
        ms[:, None, :].to_broadcast((P, KO, LL)), mybir.AluOpType.subtract)
    tp = pools.get('tapfn')
    if tp:
        tp(f'{name}_out', e16out)


def build_program(taps=()):
    taps = set(taps)
    nc = bacc.Bacc("TRN2", target_bir_lowering=False, debug=False,
                   num_devices=N_CORES)

    # ---- dram inputs ----
    din = {}

    def dram_in(nm, shape, dt=F16):
        din[nm] = nc.dram_tensor(nm, list(shape), dt, kind="ExternalInput")
        return din[nm]

    z0l16 = dram_in('z0_loc16', [P, KO, LL])
    x0l16 = dram_in('x0_loc16', [P, KO, LL])
    mask_self = dram_in('mask_self', [P, KT, LL])
    for pfx, nl in (('enc', LENC), ('dec', LDEC)):
        for w in ('wqT', 'wkT', 'wvT', 'woT'):
            dram_in(f'{pfx}_{w}', [nl, P, KO, DE])
        dram_in(f'{pfx}_w1T', [nl, 16, P, KO, LL])
        dram_in(f'{pfx}_w2T', [nl, 16, P, 16, P])
    wuT = dram_in('wuT', [64, P, KO, VC])

    # output per core: all 32000 vocab x its 256 local positions
    # [grp, ll, lt, vv*8] fp16 (one 2MB store per grp, 16KB rows)
    outp = nc.dram_tensor('outp', [8, P, 2, 8 * VC], F16,
                          kind="ExternalOutput")

    # internal dram for collectives (reused across attns; gpsimd-serialized)
    agin = nc.dram_tensor('agin', [P, 4096], F16)
    agout = nc.dram_tensor('agout', [2, P, 4096], F16)
    # tiny warm-up buffers: trigger CC channel init at t=0
    wu_in = nc.dram_tensor('wu_in', [P, 1], F16)
    wu_p_out = nc.dram_tensor('wu_p_out', [2, P, 1], F16)

    import contextlib
    with tile.TileContext(nc) as tc, contextlib.ExitStack() as octx:
        const = octx.enter_context(tc.tile_pool(name='const', bufs=1))
        ones = const.tile([P, P], F16)
        nc.vector.memset(ones[:], 1.0)
        eps128 = const.tile([P, 1], F32)
        nc.vector.memset(eps128[:], EPS)
        msk = const.tile([P, KT, LL], F16)
        nc.sync.dma_start(msk[:], mask_self[:])

        # warm up the pair CC channels immediately (the only group shape used)
        wtile = const.tile([P, 1], F16)
        nc.vector.memset(wtile[:], 0.0)
        nc.gpsimd.dma_start(wu_in[:], wtile[:])
        nc.gpsimd.collective_compute(
            "AllGather", mybir.AluOpType.bypass,
            ins=[wu_in[:]], outs=[wu_p_out[:]], replica_groups=PAIR_GROUPS)
        xf16 = const.tile([P, KO, LL], F16)

        # ================= layer phase =================
        with contextlib.ExitStack() as ctx:
            stream = ctx.enter_context(tc.tile_pool(name='stream', bufs=1))
            att = ctx.enter_context(tc.tile_pool(name='att', bufs=1))
            mlpp = ctx.enter_context(tc.tile_pool(name='mlpp', bufs=1))
            lnp = ctx.enter_context(tc.tile_pool(name='lnp', bufs=1))
            stat = ctx.enter_context(tc.tile_pool(name='stat', bufs=1))
            wqp = ctx.enter_context(tc.tile_pool(name='wqp', bufs=1))
            wkp = ctx.enter_context(tc.tile_pool(name='wkp', bufs=1))
            wvp = ctx.enter_context(tc.tile_pool(name='wvp', bufs=1))
            wop = ctx.enter_context(tc.tile_pool(name='wop', bufs=1))
            w1p = ctx.enter_context(tc.tile_pool(name='w1p', bufs=2))
            w2p = ctx.enter_context(tc.tile_pool(name='w2p', bufs=2))
            p256 = ctx.enter_context(tc.tile_pool(name='p256', bufs=2,
                                                  space='PSUM'))
            p512 = ctx.enter_context(tc.tile_pool(name='p512', bufs=4,
                                                  space='PSUM'))

            pools = dict(att=att, mlp=mlpp, lnp=lnp, p256=p256, p512=p512,
                         stat=stat, ones=ones, eps128=eps128[:],
                         wqp=wqp, wkp=wkp, wvp=wvp, wop=wop, w1p=w1p, w2p=w2p)

            def tapfn(nm, t):
                if nm not in taps:
                    return
                d = nc.dram_tensor('tap_' + nm, list(t.shape),
                                   t.dtype, kind="ExternalOutput")
                nc.sync.dma_start(d[:], t[:])
            pools['tapfn'] = tapfn

            # ======== encoder ========
            Eres = stream.tile([P, KO, LL], F32, tag='res')
            eloc = stream.tile([P, KO, LL], F16, tag='loc_a')
            nc.sync.dma_start(eloc[:], z0l16[:])

            for l in range(LENC):
                W = _load_attn_w(nc, pools, din['enc_wqT'][l],
                                 din['enc_wkT'][l], din['enc_wvT'][l],
                                 din['enc_woT'][l])
                kfull = att.tile([P, KO, L], F16, tag='kfull')
                vt16 = att.tile([P, KT, H * DA], F16, tag='vt16')
                _kv_proj_ag(nc, pools, W, eloc, agin, agout, kfull, vt16)
                _attn_core(nc, pools, Eres, eloc, W, kfull, vt16, None,
                           f'e{l}a')
                eloc = stream.tile([P, KO, LL], F16, tag='loc_b')
                _ln(nc, pools, Eres, eloc, f'e{l}ln1')
                _mlp(nc, pools, Eres, eloc, din['enc_w1T'][l],
                     din['enc_w2T'][l], f'e{l}m')
                eloc = stream.tile([P, KO, LL], F16, tag='loc_a')
                _ln(nc, pools, Eres, eloc, f'e{l}ln2')

            Zloc = stream.tile([P, KO, LL], F16, tag='zloc')
            nc.vector.tensor_copy(Zloc[:], eloc[:])

            # ======== decoder ========
            eloc = stream.tile([P, KO, LL], F16, tag='loc_a')
            nc.sync.dma_start(eloc[:], x0l16[:])

            for l in range(LDEC):
                W = _load_attn_w(nc, pools, din['dec_wqT'][l],
                                 din['dec_wkT'][l], din['dec_wvT'][l],
                                 din['dec_woT'][l])
                # self K/V + AG
                kfull_s = att.tile([P, KO, L], F16, tag='kfull')
                vt16_s = att.tile([P, KT, H * DA], F16, tag='vt16')
                _kv_proj_ag(nc, pools, W, eloc, agin, agout, kfull_s, vt16_s)
                # cross K/V + AG (overlaps self AG; weights shared)
                kfull_c = att.tile([P, KO, L], F16, tag='kfull_c')
                vt16_c = att.tile([P, KT, H * DA], F16, tag='vt16_c')
                _kv_proj_ag(nc, pools, W, Zloc, agin, agout, kfull_c, vt16_c)
                # self attention (causal)
                _attn_core(nc, pools, Eres, eloc, W, kfull_s, vt16_s, msk,
                           f'd{l}s')
                eloc = stream.tile([P, KO, LL], F16, tag='loc_b')
                _ln(nc, pools, Eres, eloc, f'd{l}ln1')
                # cross attention
                _attn_core(nc, pools, Eres, eloc, W, kfull_c, vt16_c, None,
                           f'd{l}c')
                eloc = stream.tile([P, KO, LL], F16, tag='loc_b')
                _ln(nc, pools, Eres, eloc, f'd{l}ln2')
                _mlp(nc, pools, Eres, eloc, din['dec_w1T'][l],
                     din['dec_w2T'][l], f'd{l}m')
                eloc = stream.tile([P, KO, LL], F16, tag='loc_a')
                _ln(nc, pools, Eres, eloc, f'd{l}ln3')

            # stash the final stream for the unembed phase
            nc.vector.tensor_copy(xf16[:], eloc[:])

        # ======== unembed phase (position-local: full vocab per core) =====
        with contextlib.ExitStack() as ctx:
            usb = ctx.enter_context(tc.tile_pool(name='usb', bufs=1))
            wup = ctx.enter_context(tc.tile_pool(name='wup', bufs=6))
            u512 = ctx.enter_context(tc.tile_pool(name='u512', bufs=6,
                                                  space='PSUM'))

            expu = usb.tile([P, 2, 64 * VC], F16, tag='expu')
            dacc = usb.tile([P, 2, 64], F32, tag='dacc')
            for vc in range(64):
                wut = wup.tile([P, KO, VC], F16, tag='wut')
                (nc.sync if vc % 2 == 0 else nc.scalar).dma_start(
                    wut[:], wuT[vc])
                for lt in range(2):
                    ps = u512.tile([P, VC], F32, tag='u512')
                    for k in range(KO):
                        nc.tensor.matmul(
                            ps[:], xf16[:, k, lt * P:(lt + 1) * P],
                            wut[:, k, :], start=(k == 0), stop=(k == KO - 1))
                    nc.scalar.activation(
                        expu[:, lt, vc * VC:(vc + 1) * VC], ps[:], AF.Exp,
                        accum_out=dacc[:, lt, vc:vc + 1])
            # denominator: tree-reduce the 64 per-chunk sums (local only)
            cur = dacc
            width = 64
            while width > 1:
                width //= 2
                nxt = usb.tile([P, 2, width], F32, tag=f'dt{width}')
                nc.vector.tensor_tensor(nxt[:], cur[:, :, 0:width],
                                        cur[:, :, width:2 * width],
                                        mybir.AluOpType.add)
                cur = nxt
            binv = usb.tile([P, 2, 1], F32, tag='binv')
            nc.vector.reciprocal_approx_fast(binv[:], cur[:])
            if 'deno' in taps:
                d = nc.dram_tensor('tap_deno', [P, 2, 1], F32,
                                   kind="ExternalOutput")
                nc.sync.dma_start(d[:], cur[:])

            # normalize in place (3 engines), then one big store per group
            dmae = [nc.sync, nc.gpsimd, nc.scalar]
            eng = 0
            for grp in range(8):
                for lt in range(2):
                    sl = expu[:, lt, grp * 8 * VC:(grp + 1) * 8 * VC]
                    i = eng % 3
                    eng += 1
                    if i == 0:
                        nc.vector.tensor_tensor(
                            sl, sl,
                            binv[:, lt, :].to_broadcast((P, 8 * VC)),
                            mybir.AluOpType.mult)
                    elif i == 1:
                        nc.scalar.activation(sl, sl, AF.Copy,
                                             scale=binv[:, lt, :])
                    else:
                        nc.gpsimd.tensor_tensor(
                            sl, sl,
                            binv[:, lt, :].to_broadcast((P, 8 * VC)),
                            mybir.AluOpType.mult)
                dmae[grp % 3].dma_start(
                    outp[grp], expu[:, :, grp * 8 * VC:(grp + 1) * 8 * VC])

    nc.compile()
    return nc


# ----------------------------------------------------------------------------
# host-side prep
# ----------------------------------------------------------------------------

def _to_kimaj(a):
    """[K, M] -> [128, K//128, M] with K = ko*128 + ki."""
    K, M = a.shape
    return np.ascontiguousarray(
        a.reshape(K // P, P, M).transpose(1, 0, 2))


def prep_inputs(inputs):
    f = lambda k: np.asarray(inputs[k], dtype=np.float32)
    We, Wp, Wu = f('We'), f('Wp'), f('Wu')
    x = np.asarray(inputs['x']).astype(np.int64)
    z = np.asarray(inputs['z']).astype(np.int64)

    shared = {}
    for pfx, nl in (('enc', LENC), ('dec', LDEC)):
        Wq, Wk, Wv = f(pfx + '_Wq'), f(pfx + '_Wk'), f(pfx + '_Wv')
        Wo, W1, W2 = f(pfx + '_Wo'), f(pfx + '_W1'), f(pfx + '_W2')
        wq, wk, wv, wo, w1, w2 = [], [], [], [], [], []
        for l in range(nl):
            qa = Wq[l].transpose(2, 0, 1).reshape(DE, H * DA) * (DA ** -0.5)
            ka = Wk[l].transpose(2, 0, 1).reshape(DE, H * DA)
            va = Wv[l].transpose(2, 0, 1).reshape(DE, H * DA)
            wq.append(_to_kimaj(qa))
            wk.append(_to_kimaj(ka))
            wv.append(_to_kimaj(va))
            wo.append(_to_kimaj(Wo[l].T))
            w1k = _to_kimaj(W1[l].T)          # [128, 8, 4096]
            w1.append(np.ascontiguousarray(
                w1k.reshape(P, KO, 16, LL).transpose(2, 0, 1, 3)))
            w2k = _to_kimaj(W2[l].T)          # [128, 32, 1024]
            w2.append(np.ascontiguousarray(
                w2k.reshape(P, 2, 16, 8, P).transpose(3, 1, 0, 2, 4)
                .reshape(16, P, 16, P)))
        shared[f'{pfx}_wqT'] = np.stack(wq).astype(np.float16)
        shared[f'{pfx}_wkT'] = np.stack(wk).astype(np.float16)
        shared[f'{pfx}_wvT'] = np.stack(wv).astype(np.float16)
        shared[f'{pfx}_woT'] = np.stack(wo).astype(np.float16)
        shared[f'{pfx}_w1T'] = np.stack(w1).astype(np.float16)
        shared[f'{pfx}_w2T'] = np.stack(w2).astype(np.float16)

    # full unembedding matrix, chunked: [64][128][8][500]
    wuk = _to_kimaj(Wu.T)                                  # [128, 8, 32000]
    shared['wuT'] = np.ascontiguousarray(
        wuk.reshape(P, KO, 64, VC).transpose(2, 0, 1, 3)).astype(np.float16)

    pos = Wp[:L]  # [512, 1024]
    in_maps = []
    for c in range(N_CORES):
        b, h = c // 2, c % 2
        m = dict(shared)
        for nm, tok in (('z0', z[b]), ('x0', x[b])):
            E0 = (We[tok] + pos).T.astype(np.float32)      # [1024, 512]
            E0k = E0.reshape(KO, P, L)                     # [ko, ki, p]
            loc = E0k[:, :, h * LL:(h + 1) * LL].transpose(1, 0, 2)
            m[nm + '_loc16'] = np.ascontiguousarray(loc).astype(np.float16)
        kglob = np.arange(L)[:, None]
        qglob = (h * LL + np.arange(LL))[None, :]
        msk = (kglob <= qglob).astype(np.float16)          # [512, 256]
        m['mask_self'] = np.ascontiguousarray(
            msk.reshape(KT, P, LL).transpose(1, 0, 2))
        in_maps.append(m)
    return in_maps


def assemble(results):
    """results: per-core dicts with 'outp' [8, 128, 2, 4000] fp16
    (core c covers batch c//2, positions [(c%2)*256, (c%2)*256+256))."""
    out = np.empty((4, NV, L), dtype=np.float32)
    for c, r in enumerate(results):
        b, h = c // 2, c % 2
        o = np.asarray(r['outp'], dtype=np.float32)  # [grp, ll, lt, vv]
        o = o.reshape(8, P, 2, 8 * VC).transpose(0, 3, 2, 1)  # grp, vv, lt, ll
        out[b, :, h * LL:(h + 1) * LL] = o.reshape(NV, LL)
    return out


def run(inputs, trace=False, taps=(), trace_kwargs=None):
    key = ('prog', tuple(sorted(taps)))
    if key not in _CACHE:
        _CACHE[key] = build_program(taps=taps)
    nc = _CACHE[key]
    in_maps = prep_inputs(inputs)
    res = run_bass_kernel_spmd(nc, in_maps, list(range(N_CORES)),
                               trace=trace, **(trace_kwargs or {}))
    return res


def kernel(**inputs):
    res = run(inputs, trace=False)
    return assemble(res.results)
